# revision 1
# baseline (speedup 1.0000x reference)
"""Trainium2 Bass kernel for PVT-style spatial-reduction multi-head attention.

Problem (hardcoded shapes, fp32 inputs):
  x [2, 4096, 512]; Wq [512,512]; Wconv [512,512,2,2] (OIHW, stride 2);
  LayerNorm over the conv's flattened spatial dim (M=1024); Wkv [1024,1024];
  attention with q [B,8,4096,64], k/v [B,8,512,64]; "faithful" reshape
  (out.transpose(0,1,3,2).reshape(B,-1,512)) before Wproj [512,512].

Sharding: 8 cores = (batch b in {0,1}) x (head-pair g in {0..3}).
Core (b,g) computes heads {2g, 2g+1} of batch b and writes output rows
[b, 1024g : 1024g+1024, :].  The "faithful" reshape maps head h's attention
output exactly onto output rows [512h, 512h+512), so no cross-core
communication is needed.

Each core redundantly computes the conv + LN + (its 2 heads' slice of) KV
for its batch.  All matmuls run in bf16 with fp32 PSUM accumulation;
softmax statistics, LayerNorm and softmax normalization run in fp32.
All data transposes run on the PE (transpose-mode matmul); DMA-transpose
and per-strip DMAs were the bottleneck in v1 (~480 DMA instructions,
~1 ms); this version has ~65 DMAs and measures ~110 us/iteration on HW.
"""

import sys

sys.path.insert(0, "/opt/trn_rl_repo")

import numpy as np
import ml_dtypes

import concourse.bass as bass
import concourse.bacc as bacc
import concourse.mybir as mybir
import concourse.tile as tile
from concourse.bass_utils import run_bass_kernel_spmd

F32 = mybir.dt.float32
BF16 = mybir.dt.bfloat16
NP_BF16 = ml_dtypes.bfloat16

B, N, C = 2, 4096, 512
HH, WW, SR = 64, 64, 2
NH, HD = 8, 64
M = (HH // SR) * (WW // SR)  # 1024
UN = C  # units = 512
EPS = 1e-5
N_CORES = 8

# smallbuf column map (packed [128, x] f32 bias/stat constants)
COL_BQ = 0
COL_BCONV = 1  # 4 cols
COL_GAMMA = 5  # 8 cols
COL_BKV_K = 13
COL_BKV_V = 14
SMALL_COLS = 15


def _build_module(reps=1, bench_internal=False):
    nc = bacc.Bacc("TRN2", target_bir_lowering=False, debug=False)

    # ---- per-core DRAM tensors (data differs per core, shapes identical) ----
    # bench_internal: big inputs become Internal DRAM (garbage values) so the
    # per-call PJRT input staging disappears — timing-only builds.
    KIND = "Internal" if bench_internal else "ExternalInput"
    xb16 = nc.dram_tensor("xb16", [N, C], BF16, kind=KIND).ap()
    wq = nc.dram_tensor("wq", [C, 128], BF16, kind=KIND).ap()
    wconvt = nc.dram_tensor("wconvt", [C, 4, C], BF16, kind=KIND).ap()
    wkv2 = nc.dram_tensor("wkv2", [M, 256], BF16, kind=KIND).ap()
    wproj = nc.dram_tensor("wproj", [UN, C], BF16, kind=KIND).ap()
    small = nc.dram_tensor("small", [128, SMALL_COLS], F32, kind="ExternalInput").ap()
    beta8 = nc.dram_tensor("beta8", [128, 8], BF16, kind=KIND).ap()
    bproj16 = nc.dram_tensor("bproj16", [1, C], BF16, kind=KIND).ap()
    eye128 = nc.dram_tensor("eye128", [128, 128], BF16, kind=KIND).ap()
    eye8 = nc.dram_tensor("eye8", [8, 8], F32, kind=KIND).ap()
    out = nc.dram_tensor("out", [2 * UN, C], F32, kind="ExternalOutput").ap()

    AX = mybir.AxisListType.X
    OP = mybir.AluOpType
    AF = mybir.ActivationFunctionType

    with tile.TileContext(nc) as tc:
        import contextlib

        with contextlib.ExitStack() as ctx:
            persist = ctx.enter_context(tc.tile_pool(name="persist", bufs=1))
            stage = ctx.enter_context(tc.tile_pool(name="stage", bufs=3))
            ps512 = ctx.enter_context(tc.tile_pool(name="ps512", bufs=2, space="PSUM"))
            psav = ctx.enter_context(tc.tile_pool(name="psav", bufs=2, space="PSUM"))
            pssm = ctx.enter_context(tc.tile_pool(name="pssm", bufs=1, space="PSUM"))
            pstp = ctx.enter_context(tc.tile_pool(name="pstp", bufs=3, space="PSUM"))

            for _rep in range(reps):
                # ---------------- weight / bias loads ----------------
                wq_sb = []
                for k in range(4):
                    t = persist.tile([128, 128], BF16, name=f"wq_sb{k}", tag=f"wq{k}")
                    nc.sync.dma_start(t[:], wq[128 * k : 128 * (k + 1), :])
                    wq_sb.append(t)

                wconv_sb = []  # [kt] -> [128 ic, (tap, o) 2048]
                for kt in range(4):
                    t = persist.tile([128, 2048], BF16, name=f"wconv{kt}", tag=f"wc{kt}")
                    nc.sync.dma_start(t[:], wconvt[128 * kt : 128 * (kt + 1), :, :])
                    wconv_sb.append(t)

                wkv_sb = []  # [k] -> [128 mm, 256] (cols 0:128 k-heads, 128:256 v-heads)
                for k in range(8):
                    t = persist.tile([128, 256], BF16, name=f"wkv_sb{k}", tag=f"wkv{k}")
                    nc.sync.dma_start(t[:], wkv2[128 * k : 128 * (k + 1), :])
                    wkv_sb.append(t)

                wproj_sb = []
                for ct in range(4):
                    t = persist.tile([128, 512], BF16, name=f"wproj{ct}", tag=f"wp{ct}")
                    nc.sync.dma_start(t[:], wproj[128 * ct : 128 * (ct + 1), :])
                    wproj_sb.append(t)

                small_sb = persist.tile([128, SMALL_COLS], F32, name="small_sb", tag="small")
                nc.sync.dma_start(small_sb[:], small[:, :])
                beta_sb = persist.tile([128, 8], BF16, name="beta_sb", tag="beta8")
                nc.sync.dma_start(beta_sb[:], beta8[:, :])
                bproj_sb = persist.tile([1, 512], BF16, name="bproj_sb", tag="bpj")
                nc.sync.dma_start(bproj_sb[:], bproj16[:, :])
                eye_sb = persist.tile([128, 128], BF16, name="eye_sb", tag="eye128")
                nc.sync.dma_start(eye_sb[:], eye128[:, :])
                eye8_sb = persist.tile([8, 8], F32, name="eye8_sb", tag="eye8")
                nc.sync.dma_start(eye8_sb[:], eye8[:, :])
                ones1_sb = persist.tile([1, 128], BF16, name="ones1_sb", tag="ones1")
                nc.vector.memset(ones1_sb[:], 1.0)

                bq_col = small_sb[:, COL_BQ : COL_BQ + 1]
                bkvk_col = small_sb[:, COL_BKV_K : COL_BKV_K + 1]
                bkvv_col = small_sb[:, COL_BKV_V : COL_BKV_V + 1]

                # ---------------- x: load bf16, transpose on PE ----------------
                # xt_all: [128 (c-slice), 4*4096] bf16 == x^T, c-major blocks
                xt_all = persist.tile([128, 4 * N], BF16, name="xt_all", tag="xtall")
                xt_sb = [xt_all[:, c * N : (c + 1) * N] for c in range(4)]
                xt_dst = xt_all.rearrange("p (c n) -> p c n", c=4, n=N)
                for i in range(32):
                    x16 = stage.tile([128, 512], BF16, name="x16", tag="x16")
                    nc.sync.dma_start(x16[:], xb16[128 * i : 128 * (i + 1), :])
                    tp4 = pstp.tile([128, 512], BF16, name="tp4", tag="ptp")
                    for c in range(4):
                        nc.tensor.transpose(
                            tp4[:, 128 * c : 128 * (c + 1)],
                            x16[:, 128 * c : 128 * (c + 1)],
                            eye_sb[:],
                        )
                    nc.vector.tensor_copy(
                        xt_dst[:, :, 128 * i : 128 * (i + 1)],
                        tp4.rearrange("p (c n) -> p c n", c=4, n=128),
                    )

                # ---------------- Q projection (2 heads) ----------------
                # qt_sb: [128 (2*64 d), 4096 n] bf16  == q^T for this core's heads
                qt_sb = persist.tile([128, N], BF16, name="qt_sb", tag="qt")
                for ch in range(8):
                    q_ps = ps512.tile([128, 512], F32, name="q_ps", tag="mm512")
                    for k in range(4):
                        nc.tensor.matmul(
                            q_ps[:],
                            wq_sb[k][:],
                            xt_sb[k][:, 512 * ch : 512 * (ch + 1)],
                            start=(k == 0),
                            stop=(k == 3),
                        )
                    nc.scalar.activation(
                        qt_sb[:, 512 * ch : 512 * (ch + 1)],
                        q_ps[:],
                        AF.Identity,
                        bias=bq_col,
                    )

                # ---------------- conv (full batch) ----------------
                # xc_sb[ot]: [128 (o-slice), 1024 m] f32
                xc_sb = [
                    persist.tile([128, M], F32, name=f"xc{ot}", tag=f"xc{ot}")
                    for ot in range(4)
                ]
                xt4 = [
                    xt_sb[kt].rearrange(
                        "p (i di j dj) -> p i di j dj", i=32, di=2, j=32, dj=2
                    )
                    for kt in range(4)
                ]
                for ot in range(4):
                    for half in range(2):
                        c_ps = ps512.tile([128, 512], F32, name="c_ps", tag="mm512")
                        first = True
                        for kt in range(4):
                            for di in range(2):
                                for dj in range(2):
                                    tap = 2 * di + dj
                                    rhs = xt4[kt][
                                        :, 16 * half : 16 * (half + 1), di, :, dj
                                    ]
                                    nc.tensor.matmul(
                                        c_ps[:],
                                        wconv_sb[kt][
                                            :,
                                            512 * tap + 128 * ot : 512 * tap + 128 * (ot + 1),
                                        ],
                                        rhs,
                                        start=first,
                                        stop=(kt == 3 and tap == 3),
                                    )
                                    first = False
                        nc.scalar.activation(
                            xc_sb[ot][:, 512 * half : 512 * (half + 1)],
                            c_ps[:],
                            AF.Identity,
                            bias=small_sb[:, COL_BCONV + ot : COL_BCONV + ot + 1],
                        )

                # ---------------- LayerNorm over m (free dim) ----------------
                xz_sb = [
                    persist.tile([128, M], BF16, name=f"xz{ot}", tag=f"xz{ot}")
                    for ot in range(4)
                ]
                for ot in range(4):
                    s1 = stage.tile([128, 1], F32, name="s1", tag="s1")
                    nc.vector.tensor_reduce(s1[:], xc_sb[ot][:], axis=AX, op=OP.add)
                    sq_scr = stage.tile([128, M], F32, name="sq_scr", tag="sq_scr", bufs=2)
                    s2 = stage.tile([128, 1], F32, name="s2", tag="s2")
                    # (tensor_tensor_reduce wedges the device on HW; use ACT Square)
                    nc.scalar.activation(sq_scr[:], xc_sb[ot][:], AF.Square, accum_out=s2[:])
                    mu = stage.tile([128, 1], F32, name="mu", tag="mu")
                    nc.vector.tensor_scalar_mul(mu[:], s1[:], 1.0 / M)
                    mu2 = stage.tile([128, 1], F32, name="mu2", tag="mu2")
                    nc.vector.tensor_mul(mu2[:], mu[:], mu[:])
                    ve = stage.tile([128, 1], F32, name="ve", tag="ve")
                    # s2/M + eps - mu^2  (= var + eps)
                    nc.vector.tensor_scalar(
                        out=ve[:],
                        in0=s2[:],
                        scalar1=1.0 / M,
                        scalar2=EPS,
                        op0=OP.mult,
                        op1=OP.add,
                    )
                    nc.vector.tensor_sub(ve[:], ve[:], mu2[:])
                    rv = stage.tile([128, 1], F32, name="rv", tag="rv")
                    nc.vector.reciprocal(rv[:], ve[:])
                    rs = stage.tile([128, 1], F32, name="rs", tag="rs")
                    nc.scalar.activation(rs[:], rv[:], AF.Sqrt)
                    # z = (x - mu) * rsqrt(var+eps), cast to bf16
                    nc.vector.tensor_scalar(
                        out=xz_sb[ot][:],
                        in0=xc_sb[ot][:],
                        scalar1=mu[:],
                        scalar2=rs[:],
                        op0=OP.subtract,
                        op1=OP.mult,
                    )

                # ---------------- transpose z -> [m, c] (PE) ----------------
                xzt_all = persist.tile([128, 8 * 512], BF16, name="xzt_all", tag="xztall")
                xzt_sb = [xzt_all[:, j * 512 : (j + 1) * 512] for j in range(8)]
                xzt_dst = xzt_all.rearrange("p (j c) -> p j c", j=8, c=512)
                for ot in range(4):
                    tp8a = pstp.tile([128, 512], BF16, name="tp8a", tag="ptp")
                    tp8b = pstp.tile([128, 512], BF16, name="tp8b", tag="ptp")
                    for j in range(8):
                        dst = tp8a if j < 4 else tp8b
                        nc.tensor.transpose(
                            dst[:, 128 * (j % 4) : 128 * (j % 4 + 1)],
                            xz_sb[ot][:, 128 * j : 128 * (j + 1)],
                            eye_sb[:],
                        )
                    nc.vector.tensor_copy(
                        xzt_dst[:, 0:4, 128 * ot : 128 * (ot + 1)],
                        tp8a.rearrange("p (j c) -> p j c", j=4, c=128),
                    )
                    nc.vector.tensor_copy(
                        xzt_dst[:, 4:8, 128 * ot : 128 * (ot + 1)],
                        tp8b.rearrange("p (j c) -> p j c", j=4, c=128),
                    )

                # ---------------- beta @ Wkv (bias row), gamma fold, KV ----------------
                bias_k = persist.tile([128, 1], F32, name="bias_k", tag="biask")
                bias_v = persist.tile([128, 1], F32, name="bias_v", tag="biasv")
                for which, lo, bcol, btot in (
                    ("k", 0, bkvk_col, bias_k),
                    ("v", 128, bkvv_col, bias_v),
                ):
                    bw_ps = pssm.tile([128, 1], F32, name=f"bw_ps_{which}", tag="pssm")
                    for k in range(8):
                        nc.tensor.matmul(
                            bw_ps[:],
                            wkv_sb[k][:, lo : lo + 128],
                            beta_sb[:, k : k + 1],
                            start=(k == 0),
                            stop=(k == 7),
                        )
                    nc.vector.tensor_add(btot[:], bw_ps[:], bcol)
                # fold gamma into Wkv rows (after beta rows computed)
                for k in range(8):
                    nc.vector.tensor_scalar_mul(
                        wkv_sb[k][:], wkv_sb[k][:],
                        small_sb[:, COL_GAMMA + k : COL_GAMMA + k + 1],
                    )

                # kT_sb: [128 (2 heads x 64 d), 512 cpos] bf16
                kT_sb = persist.tile([128, 512], BF16, name="kT_sb", tag="kT")
                kt_ps = ps512.tile([128, 512], F32, name="kt_ps", tag="mm512")
                for k in range(8):
                    nc.tensor.matmul(
                        kt_ps[:], wkv_sb[k][:, 0:128], xzt_sb[k][:],
                        start=(k == 0), stop=(k == 7),
                    )
                nc.scalar.activation(kT_sb[:], kt_ps[:], AF.Identity, bias=bias_k[:])

                vT_sb = persist.tile([128, 512], BF16, name="vT_sb", tag="vT")
                vt_ps = ps512.tile([128, 512], F32, name="vt_ps", tag="mm512")
                for k in range(8):
                    nc.tensor.matmul(
                        vt_ps[:], wkv_sb[k][:, 128:256], xzt_sb[k][:],
                        start=(k == 0), stop=(k == 7),
                    )
                nc.scalar.activation(vT_sb[:], vt_ps[:], AF.Identity, bias=bias_v[:])

                # v_aug[p][mt]: [128 cpos, 72] bf16 (cols 64:72 = ones; the AV
                # matmul then yields 8 identical denominator rows at psum
                # partitions 64:72, which are legally mask-accumulated below)
                vaug_sb = []
                for p in range(2):
                    row = []
                    for mt in range(4):
                        t = persist.tile(
                            [128, 72], BF16, name=f"vaug{p}_{mt}", tag=f"va{p}{mt}"
                        )
                        nc.vector.memset(t[:, 64:72], 1.0)
                        tp = pstp.tile([128, 64], BF16, name="tpv", tag="ptp")
                        nc.tensor.transpose(
                            tp[:],
                            vT_sb[64 * p : 64 * (p + 1), 128 * mt : 128 * (mt + 1)],
                            eye_sb[64 * p : 64 * (p + 1), 64 * p : 64 * (p + 1)],
                        )
                        nc.vector.tensor_copy(t[:, 0:64], tp[:])
                        row.append(t)
                    vaug_sb.append(row)

                # ---------------- attention ----------------
                # avT_sb[p]: [64 d, 4096 n] bf16 (unnormalized AV^T)
                # den_sb[p]: [8 chunk, 512 n-in-chunk] f32 (softmax denominators)
                avT_sb = [
                    persist.tile([64, N], BF16, name=f"avT{p}", tag=f"avT{p}")
                    for p in range(2)
                ]
                den_sb = [
                    persist.tile([8, 512], F32, name=f"den{p}", tag=f"den{p}")
                    for p in range(2)
                ]
                for p in range(2):
                    nc.vector.memset(den_sb[p][:], 0.0)
                for ch in range(8):
                    # S^T matmuls: interleave pairs so K=64 row-groups (0/64)
                    # run concurrently in the PE array
                    phat = {0: [], 1: []}
                    for mt in range(4):
                        for p in range(2):
                            s_ps = ps512.tile([128, 512], F32, name="s_ps", tag="mm512")
                            nc.tensor.matmul(
                                s_ps[:],
                                kT_sb[64 * p : 64 * (p + 1), 128 * mt : 128 * (mt + 1)],
                                qt_sb[64 * p : 64 * (p + 1), 512 * ch : 512 * (ch + 1)],
                                start=True,
                                stop=True,
                            )
                            ph = stage.tile(
                                [128, 512], BF16, name="phat", tag="phat", bufs=8
                            )
                            nc.scalar.activation(ph[:], s_ps[:], AF.Exp, scale=0.125)
                            phat[p].append(ph)
                    for p in range(2):
                        av_ps = psav.tile([72, 512], F32, name="av_ps", tag="psav")
                        for mt in range(4):
                            nc.tensor.matmul(
                                av_ps[:],
                                vaug_sb[p][mt][:],
                                phat[p][mt][:],
                                start=(mt == 0),
                                stop=(mt == 3),
                            )
                        nc.scalar.activation(
                            avT_sb[p][:, 512 * ch : 512 * (ch + 1)],
                            av_ps[0:64, :],
                            AF.Copy,
                        )
                        tmp8 = stage.tile([8, 512], F32, name="tmp8", tag="tmp8", bufs=2)
                        nc.vector.tensor_scalar_mul(
                            tmp8[:], av_ps[64:72, :], eye8_sb[:, ch : ch + 1]
                        )
                        nc.vector.tensor_add(den_sb[p][:], den_sb[p][:], tmp8[:])

                # ---------------- denominators: transpose + reciprocal ----------------
                # recipT_sb[p][ct]: [128 (n within 128-slice), 8 (chunk)] f32
                recipT_sb = []
                for p in range(2):
                    row = []
                    for ct in range(4):
                        dt_ps = pssm.tile([128, 8], F32, name="dt_ps", tag="pssm")
                        nc.tensor.transpose(
                            dt_ps[:],
                            den_sb[p][:, 128 * ct : 128 * (ct + 1)],
                            eye8_sb[:],
                        )
                        t = persist.tile(
                            [128, 8], F32, name=f"recT{p}_{ct}", tag=f"rc{p}{ct}"
                        )
                        nc.vector.reciprocal(t[:], dt_ps[:])
                        row.append(t)
                    recipT_sb.append(row)

                # ---------------- AV transpose (PE) + normalize into out2dT ----------------
                # out2dT[p][ct]: [128 c, 512 r_local] bf16 where r_local = 8*d + s
                out2dT = []
                for p in range(2):
                    row = []
                    for ct in range(4):
                        t = persist.tile(
                            [128, 512], BF16, name=f"o2dT{p}_{ct}", tag=f"o2{p}{ct}"
                        )
                        row.append(t)
                    out2dT.append(row)
                for p in range(2):
                    for i in range(32):
                        s, ct = i // 4, i % 4
                        tp = pstp.tile([128, 64], BF16, name="tpav", tag="ptp")
                        nc.tensor.transpose(
                            tp[:],
                            avT_sb[p][:, 128 * i : 128 * (i + 1)],
                            eye_sb[0:64, 0:64],
                        )
                        o3 = out2dT[p][ct].rearrange("p (d s) -> p d s", d=64, s=8)
                        nc.vector.tensor_scalar_mul(
                            o3[:, :, s], tp[:], recipT_sb[p][ct][:, s : s + 1]
                        )

                # ---------------- projection + output ----------------
                for p in range(2):
                    for rt in range(4):
                        pr_ps = ps512.tile([128, 512], F32, name="pr_ps", tag="mm512")
                        for ct in range(4):
                            nc.tensor.matmul(
                                pr_ps[:],
                                out2dT[p][ct][:, 128 * rt : 128 * (rt + 1)],
                                wproj_sb[ct][:],
                                start=(ct == 0),
                                stop=False,
                            )
                        nc.tensor.matmul(
                            pr_ps[:], ones1_sb[:], bproj_sb[:], start=False, stop=True
                        )
                        of = stage.tile([128, 512], F32, name="of", tag="of", bufs=3)
                        nc.scalar.activation(of[:], pr_ps[:], AF.Copy)
                        r0 = 512 * p + 128 * rt
                        nc.sync.dma_start(out[r0 : r0 + 128, :], of[:])

    nc.compile()
    return nc


_NC_CACHE = None


def _get_module():
    global _NC_CACHE
    if _NC_CACHE is None:
        _NC_CACHE = _build_module()
    return _NC_CACHE


def _prep_core_inputs(inputs):
    """Host-side sharding: slice / transpose / cast weights, build 8 in_maps."""
    x = np.asarray(inputs["x"], np.float32)
    Wq = np.asarray(inputs["Wq"], np.float32)
    bq = np.asarray(inputs["bq"], np.float32)
    Wconv = np.asarray(inputs["Wconv"], np.float32)
    bconv = np.asarray(inputs["bconv"], np.float32)
    gamma = np.asarray(inputs["gamma"], np.float32)
    beta = np.asarray(inputs["beta"], np.float32)
    Wkv = np.asarray(inputs["Wkv"], np.float32)
    bkv = np.asarray(inputs["bkv"], np.float32)
    Wproj = np.asarray(inputs["Wproj"], np.float32)
    bproj = np.asarray(inputs["bproj"], np.float32)

    # Wconv [O, I, 2, 2] -> [I, tap, O] bf16
    wconvt = np.ascontiguousarray(
        Wconv.transpose(1, 2, 3, 0).reshape(C, 4, C)
    ).astype(NP_BF16)
    eye128 = np.eye(128, dtype=np.float32).astype(NP_BF16)
    eye8 = np.eye(8, dtype=np.float32)
    beta8 = np.ascontiguousarray(beta.reshape(8, 128).T).astype(NP_BF16)
    wproj16 = np.ascontiguousarray(Wproj).astype(NP_BF16)
    bproj16 = np.ascontiguousarray(bproj.reshape(1, C)).astype(NP_BF16)
    x16 = [np.ascontiguousarray(x[b]).astype(NP_BF16) for b in range(B)]

    in_maps = []
    for core in range(N_CORES):
        b, g = divmod(core, 4)
        ucols = slice(128 * g, 128 * (g + 1))
        vcols = slice(512 + 128 * g, 512 + 128 * (g + 1))
        small = np.zeros((128, SMALL_COLS), np.float32)
        small[:, COL_BQ] = bq[ucols]
        small[:, COL_BCONV : COL_BCONV + 4] = bconv.reshape(4, 128).T
        small[:, COL_GAMMA : COL_GAMMA + 8] = gamma.reshape(8, 128).T
        small[:, COL_BKV_K] = bkv[ucols]
        small[:, COL_BKV_V] = bkv[vcols]
        wkv2 = np.concatenate([Wkv[:, ucols], Wkv[:, vcols]], axis=1)
        in_maps.append(
            {
                "xb16": x16[b],
                "wq": np.ascontiguousarray(Wq[:, ucols]).astype(NP_BF16),
                "wconvt": wconvt,
                "wkv2": np.ascontiguousarray(wkv2).astype(NP_BF16),
                "wproj": wproj16,
                "small": small,
                "beta8": beta8,
                "bproj16": bproj16,
                "eye128": eye128,
                "eye8": eye8,
            }
        )
    return in_maps


def run_spmd(inputs, **kwargs):
    """Run the SPMD kernel; returns (full_output, BassKernelResults)."""
    nc = _get_module()
    in_maps = _prep_core_inputs(inputs)
    res = run_bass_kernel_spmd(nc, in_maps, core_ids=list(range(N_CORES)), **kwargs)
    full = np.empty((B, N, C), np.float32)
    for core in range(N_CORES):
        b, g = divmod(core, 4)
        full[b, 1024 * g : 1024 * (g + 1), :] = res.results[core]["out"]
    return full, res


def kernel(**inputs) -> np.ndarray:
    full, _ = run_spmd(inputs)
    return full



# revision 9
# speedup vs baseline: 1.0437x; 1.0437x over previous
"""Trainium2 Bass kernel for PVT-style spatial-reduction multi-head attention.

Problem (hardcoded shapes, fp32 inputs):
  x [2, 4096, 512]; Wq [512,512]; Wconv [512,512,2,2] (OIHW, stride 2);
  LayerNorm over the conv's flattened spatial dim (M=1024); Wkv [1024,1024];
  attention with q [B,8,4096,64], k/v [B,8,512,64]; "faithful" reshape
  (out.transpose(0,1,3,2).reshape(B,-1,512)) before Wproj [512,512].

Sharding: 8 cores = (batch b in {0,1}) x (head-pair g in {0..3}).

v3 design vs the v2 baseline (113 us):
 - x is sent host-side in a tap-expanded transposed layout Xp[ic, (m, tap)]
   (n = 128i + 64di + 2j + dj; m = 32i + j; tap = 2di + dj), so no on-chip
   x transpose. Q consumes Xp directly; the resulting within-chunk column
   permutation of q (and of the attention output) is absorbed into a
   host-side row permutation of Wproj.
 - The stride-2 2x2 VALID conv is non-overlapping, computed TRANSPOSED
   (xcT [m, o]) and m-sharded 4 ways across the cores of a batch; the full
   xcT is restored with a DRAM AllGather (bf16, 256KB -> 1MB per core).
   Per-core m-block assignment is encoded by host-side rotation of Xp's
   m-blocks (SPMD program identical across cores); the output-row
   permutation this induces is undone on the host after the run.
 - LayerNorm is folded algebraically: gamma into Wkv rows (host), beta+bkv
   into a bias row s (host), so kv = rs_c*(xcT @ Wkv' - mu_c t + s*sqrtve_c)
   with the mu/s terms as K=1 rank-1 PE matmuls into the same PSUM group.
   The per-position scale rs_c = rsqrt(var_c+eps) folds into the softmax
   exp scale (k side, per-partition AP scale) and the vaug scale (v side).
   Stats (sum x, sum x^2) come from ones-vector PE matmuls + a 4KB DRAM
   AllReduce; rsqrt is computed as exp(-0.5*ln(v)) so the ACT engine stays
   on the one activation table that holds exp/ln/copy/square/identity
   (no 1283ns table reloads).
 - Softmax denominators are 1-column PE matmuls (phat^T @ ones) written
   directly in the transposed [n, ch] layout (no DVE accumulate pass, no
   ones-augmented AV columns).
 - Engine balance: ACT does exp (+ tiny ln/exp stats, kv/proj epilogues),
   gpsimd does the AV PSUM->SBUF copies, DVE does Q epilogue + AV
   normalize + small stats math.
"""

import sys

sys.path.insert(0, "/opt/trn_rl_repo")

import math

import numpy as np
import ml_dtypes

import concourse.bass as bass
import concourse.bacc as bacc
import concourse.mybir as mybir
import concourse.tile as tile
from concourse.bass_utils import run_bass_kernel_spmd

F32 = mybir.dt.float32
BF16 = mybir.dt.bfloat16
NP_BF16 = ml_dtypes.bfloat16

B, N, C = 2, 4096, 512
NH, HD, SR = 8, 64, 2
M = 1024
EPS = 1e-5
N_CORES = 8
GROUPS = [[0, 1, 2, 3], [4, 5, 6, 7]]

# rows16 row map ([8, 512] bf16 host constants)
ROW_BCONV = 0
ROW_BPROJ = 1
ROW_TK = 2
ROW_TV = 3
ROW_SK = 4
ROW_SV = 5
ROW_ONES = 6


def _build_module(reps=1, bench_internal=False):
    nc = bacc.Bacc("TRN2", target_bir_lowering=False, debug=False)

    # ---- per-core DRAM tensors (data differs per core, shapes identical) ----
    KIND = "Internal" if bench_internal else "ExternalInput"
    xp = nc.dram_tensor("xp", [C, N], BF16, kind=KIND).ap()
    wq = nc.dram_tensor("wq", [C, 128], BF16, kind=KIND).ap()
    wconvt = nc.dram_tensor("wconvt", [C, 4, C], BF16, kind=KIND).ap()
    wkv2 = nc.dram_tensor("wkv2", [M, 256], BF16, kind=KIND).ap()
    wproj = nc.dram_tensor("wproj", [C, C], BF16, kind=KIND).ap()
    small = nc.dram_tensor("small", [128, 2], F32, kind="ExternalInput").ap()
    rows16 = nc.dram_tensor("rows16", [1, 8 * C], BF16, kind=KIND).ap()
    eye128 = nc.dram_tensor("eye128", [128, 128], BF16, kind=KIND).ap()
    eyef = nc.dram_tensor("eyef", [8, 8], F32, kind=KIND).ap()
    out = nc.dram_tensor("out", [2 * C, C], F32, kind="ExternalOutput").ap()

    AX = mybir.AxisListType.X
    OP = mybir.AluOpType
    AF = mybir.ActivationFunctionType
    LN8 = math.log(0.125)

    with tile.TileContext(nc) as tc:
        import contextlib

        with contextlib.ExitStack() as ctx:
            persist = ctx.enter_context(tc.tile_pool(name="persist", bufs=1))
            stage = ctx.enter_context(tc.tile_pool(name="stage", bufs=3))
            ps512 = ctx.enter_context(tc.tile_pool(name="ps512", bufs=3, space="PSUM"))
            psav = ctx.enter_context(tc.tile_pool(name="psav", bufs=2, space="PSUM"))
            psden = ctx.enter_context(tc.tile_pool(name="psden", bufs=1, space="PSUM"))
            pstp = ctx.enter_context(tc.tile_pool(name="pstp", bufs=2, space="PSUM"))
            dram = ctx.enter_context(tc.tile_pool(name="dram", bufs=2, space="DRAM"))

            for _rep in range(reps):
                # ---------------- weight / const loads ----------------
                wq_sb = []
                for k in range(4):
                    t = persist.tile([128, 128], BF16, name=f"wq_sb{k}", tag=f"wq{k}")
                    nc.sync.dma_start(t[:], wq[128 * k : 128 * (k + 1), :])
                    wq_sb.append(t)

                wconv_sb = []  # [ic_t] -> [128 ic, (tap 4, o 512)]
                for kt in range(4):
                    t = persist.tile([128, 2048], BF16, name=f"wconv{kt}", tag=f"wc{kt}")
                    nc.sync.dma_start(t[:], wconvt[128 * kt : 128 * (kt + 1), :, :])
                    wconv_sb.append(t)

                wkv_sb = []  # [mt] -> [128 m, 256] (cols 0:128 k, 128:256 v)
                for k in range(8):
                    t = persist.tile([128, 256], BF16, name=f"wkv_sb{k}", tag=f"wkv{k}")
                    nc.sync.dma_start(t[:], wkv2[128 * k : 128 * (k + 1), :])
                    wkv_sb.append(t)

                wproj_sb = []
                for ct in range(4):
                    t = persist.tile([128, 512], BF16, name=f"wproj{ct}", tag=f"wp{ct}")
                    nc.sync.dma_start(t[:], wproj[128 * ct : 128 * (ct + 1), :])
                    wproj_sb.append(t)

                small_sb = persist.tile([128, 2], F32, name="small_sb", tag="small")
                nc.sync.dma_start(small_sb[:], small[:, :])
                rows_sb = persist.tile([1, 4096], BF16, name="rows_sb", tag="rows16")
                nc.sync.dma_start(rows_sb[:], rows16[:, :])

                def crow(r, n=512):
                    return rows_sb[0:1, 512 * r : 512 * r + n]
                eye_sb = persist.tile([128, 128], BF16, name="eye_sb", tag="eye128")
                nc.sync.dma_start(eye_sb[:], eye128[:, :])
                eyef_sb = persist.tile([8, 8], F32, name="eyef_sb", tag="eyef")
                nc.sync.dma_start(eyef_sb[:], eyef[:, :])
                onec_sb = persist.tile([128, 1], BF16, name="onec_sb", tag="onec")
                nc.vector.memset(onec_sb[:], 1.0)
                ln8_sb = persist.tile([128, 1], F32, name="ln8_sb", tag="ln8")
                nc.vector.memset(ln8_sb[:], LN8)

                bq_col = small_sb[:, 0:1]

                # ---------------- x load (conv slices first) ----------------
                xp_sb = []  # [ic_t] -> [128 ic, 4096 (m,tap)]
                for kt in range(4):
                    t = persist.tile([128, N], BF16, name=f"xp_sb{kt}", tag=f"xp{kt}")
                    xp_sb.append(t)
                for kt in range(4):
                    nc.sync.dma_start(
                        xp_sb[kt][:, 0:1024], xp[128 * kt : 128 * (kt + 1), 0:1024]
                    )
                for kt in range(4):
                    nc.sync.dma_start(
                        xp_sb[kt][:, 1024:4096], xp[128 * kt : 128 * (kt + 1), 1024:4096]
                    )
                xp4 = [t.rearrange("p (m tap) -> p m tap", m=M, tap=4) for t in xp_sb]

                # ---------------- conv (local m-blocks 0,1), stats ----------------
                xcl_sb = []  # local conv out [128 m, 512 o] bf16
                sq_sb = []

                for l in range(2):
                    c_ps = ps512.tile([128, 512], F32, name="c_ps", tag="mm512")
                    first = True
                    for kt in range(4):
                        for tap in range(4):
                            nc.tensor.matmul(
                                c_ps[:],
                                xp4[kt][:, 128 * l : 128 * (l + 1), tap],
                                wconv_sb[kt][:, 512 * tap : 512 * (tap + 1)],
                                start=first,
                                stop=False,
                            )
                            first = False
                    # += ones (x) bconv  (rank-1 bias over m partitions)
                    nc.tensor.matmul(
                        c_ps[:], crow(ROW_ONES, 128), crow(ROW_BCONV),
                        start=False, stop=True,
                    )
                    xcl = persist.tile([128, 512], BF16, name=f"xcl{l}", tag=f"xcl{l}")
                    nc.vector.tensor_copy(xcl[:], c_ps[:])
                    sq = persist.tile([128, 512], BF16, name=f"sq{l}", tag=f"sql{l}")
                    nc.scalar.activation(sq[:], c_ps[:], AF.Square)
                    xcl_sb.append(xcl)
                    sq_sb.append(sq)
                sx_ps = ps512.tile([1, 512], F32, name="sx_ps", tag="mm512")
                sq_ps = ps512.tile([1, 512], F32, name="sq_ps", tag="mm512")
                for l in range(2):
                    nc.tensor.matmul(
                        sx_ps[:], onec_sb[:], xcl_sb[l][:],
                        start=(l == 0), stop=(l == 1),
                    )
                    nc.tensor.matmul(
                        sq_ps[:], onec_sb[:], sq_sb[l][:],
                        start=(l == 0), stop=(l == 1),
                    )
                stats_sb = persist.tile([1, 1024], F32, name="stats_sb", tag="stats")
                nc.vector.tensor_copy(stats_sb[0:1, 0:512], sx_ps[:])
                nc.vector.tensor_copy(stats_sb[0:1, 512:1024], sq_ps[:])

                # ---------------- collectives ----------------
                xc_bnc_in = dram.tile([256, 512], BF16, name="xc_bnc_in")
                xc_bnc_out = dram.tile([1024, 512], BF16, name="xc_bnc_out")
                st_bnc_in = dram.tile([1, 1024], F32, name="st_bnc_in")
                st_bnc_out = dram.tile([1, 1024], F32, name="st_bnc_out")
                for l in range(2):
                    nc.gpsimd.dma_start(
                        xc_bnc_in[128 * l : 128 * (l + 1), :], xcl_sb[l][:]
                    )
                nc.gpsimd.dma_start(st_bnc_in[:], stats_sb[:])
                nc.gpsimd.collective_compute(
                    "AllGather",
                    OP.bypass,
                    replica_groups=GROUPS,
                    ins=[xc_bnc_in.opt()],
                    outs=[xc_bnc_out.opt()],
                )
                nc.gpsimd.collective_compute(
                    "AllReduce",
                    OP.add,
                    replica_groups=GROUPS,
                    ins=[st_bnc_in.opt()],
                    outs=[st_bnc_out.opt()],
                )
                srow_sb = persist.tile([1, 1024], F32, name="srow_sb", tag="srow")
                nc.sync.dma_start(srow_sb[:], st_bnc_out[:])
                sx_row = srow_sb[0:1, 0:512]
                sq_row = srow_sb[0:1, 512:1024]
                xcg_sb = []
                for k in range(8):
                    t = persist.tile([128, 512], BF16, name=f"xcg{k}", tag=f"xcg{k}")
                    nc.sync.dma_start(t[:], xc_bnc_out[128 * k : 128 * (k + 1), :])
                    xcg_sb.append(t)

                # ---------------- Q projection (overlaps collectives) ----------------
                qt_sb = persist.tile([128, N], BF16, name="qt_sb", tag="qt")
                for ch in range(8):
                    q_ps = ps512.tile([128, 512], F32, name="q_ps", tag="mm512")
                    for k in range(4):
                        nc.tensor.matmul(
                            q_ps[:],
                            wq_sb[k][:],
                            xp4[k][:, 128 * ch : 128 * (ch + 1), :],
                            start=(k == 0),
                            stop=(k == 3),
                        )
                    nc.vector.tensor_scalar_add(
                        qt_sb[:, 512 * ch : 512 * (ch + 1)], q_ps[:], bq_col
                    )

                # ---------------- stats math ----------------
                murow = stage.tile([1, 512], F32, name="murow", tag="murow", bufs=1)
                nc.vector.tensor_scalar_mul(murow[:], sx_row, 1.0 / M)
                negmu16 = persist.tile([1, 512], BF16, name="negmu16", tag="negmu")
                nc.vector.tensor_scalar_mul(negmu16[:], sx_row, -1.0 / M)
                verow = persist.tile([1, 512], F32, name="verow", tag="verow")
                nc.vector.tensor_scalar(
                    out=verow[:], in0=sq_row,
                    scalar1=1.0 / M, scalar2=EPS, op0=OP.mult, op1=OP.add,
                )
                mu2 = stage.tile([1, 512], F32, name="mu2", tag="mu2", bufs=1)
                nc.vector.tensor_mul(mu2[:], murow[:], murow[:])
                nc.vector.tensor_sub(verow[:], verow[:], mu2[:])
                # sqrtve row (bf16) = exp(0.5 ln ve)
                lrow = stage.tile([1, 512], F32, name="lrow", tag="lrow", bufs=1)
                nc.scalar.activation(lrow[:], verow[:], AF.Ln)
                sqve16 = persist.tile([1, 512], BF16, name="sqve16", tag="sqve")
                nc.scalar.activation(sqve16[:], lrow[:], AF.Exp, scale=0.5)
                # columns: ve -> [128, 4] via PE transpose, then exp/ln scales
                vecol_ps = ps512.tile([128, 4], F32, name="vecol_ps", tag="mm512")
                for j in range(4):
                    nc.tensor.transpose(
                        vecol_ps[:, j : j + 1],
                        verow[:, 128 * j : 128 * (j + 1)],
                        eyef_sb[0:1, 0:1],
                    )
                lcol = stage.tile([128, 4], F32, name="lcol", tag="lcol", bufs=1)
                nc.scalar.activation(lcol[:], vecol_ps[:], AF.Ln)
                esc_col = persist.tile([128, 4], F32, name="esc_col", tag="esc")
                nc.scalar.activation(esc_col[:], lcol[:], AF.Exp, scale=-0.5, bias=ln8_sb[:])
                vsc_col = persist.tile([128, 4], F32, name="vsc_col", tag="vsc")
                nc.scalar.activation(vsc_col[:], lcol[:], AF.Exp, scale=-0.5)

                # ---------------- KV ----------------
                kT_sb = persist.tile([128, 512], BF16, name="kT_sb", tag="kT")
                vT_sb = persist.tile([128, 512], BF16, name="vT_sb", tag="vT")
                for which, lo, t_row, s_row, dst in (
                    ("k", 0, ROW_TK, ROW_SK, kT_sb),
                    ("v", 128, ROW_TV, ROW_SV, vT_sb),
                ):
                    kv_ps = ps512.tile([128, 512], F32, name="kv_ps", tag="mm512")
                    for k in range(8):
                        nc.tensor.matmul(
                            kv_ps[:], wkv_sb[k][:, lo : lo + 128], xcg_sb[k][:],
                            start=(k == 0), stop=False,
                        )
                    nc.tensor.matmul(
                        kv_ps[:], crow(t_row, 128), negmu16[:],
                        start=False, stop=False,
                    )
                    nc.tensor.matmul(
                        kv_ps[:], crow(s_row, 128), sqve16[:],
                        start=False, stop=True,
                    )
                    nc.scalar.activation(dst[:], kv_ps[:], AF.Copy)

                # vaug[p][mt]: [128 c, 64 d] bf16, v^T with rs folded
                vaug_sb = []
                for p in range(2):
                    row = []
                    for mt in range(4):
                        t = persist.tile(
                            [128, 64], BF16, name=f"vaug{p}_{mt}", tag=f"va{p}{mt}"
                        )
                        tp = pstp.tile([128, 64], BF16, name="tpv", tag="ptp")
                        nc.tensor.transpose(
                            tp[:],
                            vT_sb[64 * p : 64 * (p + 1), 128 * mt : 128 * (mt + 1)],
                            eye_sb[64 * p : 64 * (p + 1), 64 * p : 64 * (p + 1)],
                        )
                        nc.vector.tensor_scalar_mul(
                            t[:], tp[:], vsc_col[:, mt : mt + 1]
                        )
                        row.append(t)
                    vaug_sb.append(row)

                # ---------------- attention ----------------
                avT_sb = [
                    persist.tile([64, N], BF16, name=f"avT{p}", tag=f"avT{p}")
                    for p in range(2)
                ]
                den_all = psden.tile([128, 64], F32, name="den_all", tag="den")
                den_ps = [den_all[:, 32 * p : 32 * (p + 1)] for p in range(2)]
                for ch in range(8):
                    phat = {0: [], 1: []}
                    for mt in range(4):
                        for p in range(2):
                            s_ps = ps512.tile([128, 512], F32, name="s_ps", tag="mm512")
                            nc.tensor.matmul(
                                s_ps[:],
                                kT_sb[64 * p : 64 * (p + 1), 128 * mt : 128 * (mt + 1)],
                                qt_sb[64 * p : 64 * (p + 1), 512 * ch : 512 * (ch + 1)],
                                start=True,
                                stop=True,
                            )
                            ph = stage.tile(
                                [128, 512], BF16, name="phat", tag="phat", bufs=8
                            )
                            nc.scalar.activation(
                                ph[:], s_ps[:], AF.Exp, scale=esc_col[:, mt : mt + 1]
                            )
                            phat[p].append(ph)
                    for p in range(2):
                        av_ps = psav.tile([64, 512], F32, name="av_ps", tag="psav")
                        for mt in range(4):
                            nc.tensor.matmul(
                                av_ps[:],
                                vaug_sb[p][mt][:],
                                phat[p][mt][:],
                                start=(mt == 0),
                                stop=(mt == 3),
                            )
                        nc.vector.tensor_copy(
                            avT_sb[p][:, 512 * ch : 512 * (ch + 1)], av_ps[:]
                        )
                        # denominators, directly transposed: den[n, ch]
                        for ct in range(4):
                            for mt in range(4):
                                nc.tensor.matmul(
                                    den_ps[p][:, 8 * ct + ch : 8 * ct + ch + 1],
                                    phat[p][mt][:, 128 * ct : 128 * (ct + 1)],
                                    onec_sb[:],
                                    start=(mt == 0),
                                    stop=(mt == 3),
                                )

                # reciprocals of denominators
                recT = []
                for p in range(2):
                    row = []
                    for ct in range(4):
                        t = persist.tile(
                            [128, 8], F32, name=f"recT{p}_{ct}", tag=f"rc{p}{ct}"
                        )
                        nc.vector.reciprocal(t[:], den_ps[p][:, 8 * ct : 8 * (ct + 1)])
                        row.append(t)
                    recT.append(row)

                # ---------------- AV transpose (PE) + normalize ----------------
                out2dT = []
                for p in range(2):
                    row = []
                    for ct in range(4):
                        t = persist.tile(
                            [128, 512], BF16, name=f"o2dT{p}_{ct}", tag=f"o2{p}{ct}"
                        )
                        row.append(t)
                    out2dT.append(row)
                for p in range(2):
                    for i in range(32):
                        s, ct = i // 4, i % 4
                        tp = pstp.tile([128, 64], BF16, name="tpav", tag="ptp")
                        nc.tensor.transpose(
                            tp[:],
                            avT_sb[p][:, 128 * i : 128 * (i + 1)],
                            eye_sb[0:64, 0:64],
                        )
                        o3 = out2dT[p][ct].rearrange("p (d s) -> p d s", d=64, s=8)
                        nc.vector.tensor_scalar_mul(
                            o3[:, :, s], tp[:], recT[p][ct][:, s : s + 1]
                        )

                # ---------------- projection + output ----------------
                for p in range(2):
                    for rt in range(4):
                        pr_ps = ps512.tile([128, 512], F32, name="pr_ps", tag="mm512")
                        for ct in range(4):
                            nc.tensor.matmul(
                                pr_ps[:],
                                out2dT[p][ct][:, 128 * rt : 128 * (rt + 1)],
                                wproj_sb[ct][:],
                                start=(ct == 0),
                                stop=False,
                            )
                        nc.tensor.matmul(
                            pr_ps[:], crow(ROW_ONES, 128), crow(ROW_BPROJ),
                            start=False, stop=True,
                        )
                        of = stage.tile([128, 512], F32, name="of", tag="of", bufs=3)
                        nc.scalar.activation(of[:], pr_ps[:], AF.Copy)
                        r0 = 512 * p + 128 * rt
                        nc.sync.dma_start(out[r0 : r0 + 128, :], of[:])

    nc.compile()
    return nc


_NC_CACHE = None


def _get_module():
    global _NC_CACHE
    if _NC_CACHE is None:
        _NC_CACHE = _build_module()
    return _NC_CACHE


def _prep_core_inputs(inputs):
    """Host-side sharding: layout/permute/cast weights, build 8 in_maps."""
    x = np.asarray(inputs["x"], np.float32)
    Wq = np.asarray(inputs["Wq"], np.float32)
    bq = np.asarray(inputs["bq"], np.float32)
    Wconv = np.asarray(inputs["Wconv"], np.float32)
    bconv = np.asarray(inputs["bconv"], np.float32)
    gamma = np.asarray(inputs["gamma"], np.float32)
    beta = np.asarray(inputs["beta"], np.float32)
    Wkv = np.asarray(inputs["Wkv"], np.float32)
    bkv = np.asarray(inputs["bkv"], np.float32)
    Wproj = np.asarray(inputs["Wproj"], np.float32)
    bproj = np.asarray(inputs["bproj"], np.float32)

    # Xp: [ic, (m, tap)]; n = 128i + 64di + 2j + dj, m = 32i+j, tap = 2di+dj
    xp_g = []
    for b in range(B):
        xt = x[b].T.reshape(C, 32, 2, 32, 2)  # [ic, i, di, j, dj]
        xt = np.ascontiguousarray(
            xt.transpose(0, 1, 3, 2, 4).reshape(C, 8, 512)  # [ic, blk, rest]
        )
        xp_g.append(xt)

    wconvt = np.ascontiguousarray(
        Wconv.transpose(1, 2, 3, 0).reshape(C, 4, C)
    ).astype(NP_BF16)

    wkvp = gamma[:, None] * Wkv
    s_full = beta @ Wkv + bkv

    # Wproj row permutation: u' = 128 i' + 4 j + 2 di + dj -> n' = 128 i' + 64 di + 2 j + dj
    up = np.arange(C)
    i_, j_ = up // 128, (up % 128) // 4
    di, dj = (up % 4) // 2, up % 2
    nprime = 128 * i_ + 64 * di + 2 * j_ + dj
    wproj_perm = np.ascontiguousarray(Wproj[nprime, :]).astype(NP_BF16)

    eye128 = np.eye(128, dtype=np.float32).astype(NP_BF16)
    eyef = np.eye(8, dtype=np.float32)

    in_maps = []
    for core in range(N_CORES):
        b, g = divmod(core, 4)
        kcols = slice(128 * g, 128 * (g + 1))
        vcols = slice(512 + 128 * g, 512 + 128 * (g + 1))
        # local m-block rotation: local block l = global (l + 2g) % 8
        xp_loc = np.ascontiguousarray(
            np.roll(xp_g[b], -2 * g, axis=1).reshape(C, N)
        ).astype(NP_BF16)
        small = np.zeros((128, 2), np.float32)
        small[:, 0] = bq[kcols]
        rows16 = np.zeros((8, C), np.float32)
        rows16[ROW_BCONV] = bconv
        rows16[ROW_BPROJ] = bproj
        rows16[ROW_TK, 0:128] = wkvp[:, kcols].sum(0)
        rows16[ROW_TV, 0:128] = wkvp[:, vcols].sum(0)
        rows16[ROW_SK, 0:128] = s_full[kcols]
        rows16[ROW_SV, 0:128] = s_full[vcols]
        rows16[ROW_ONES] = 1.0
        wkv2 = np.concatenate([wkvp[:, kcols], wkvp[:, vcols]], axis=1)
        in_maps.append(
            {
                "xp": xp_loc,
                "wq": np.ascontiguousarray(Wq[:, kcols]).astype(NP_BF16),
                "wconvt": wconvt,
                "wkv2": np.ascontiguousarray(wkv2).astype(NP_BF16),
                "wproj": wproj_perm,
                "small": small,
                "rows16": rows16.reshape(1, 8 * C).astype(NP_BF16),
                "eye128": eye128,
                "eyef": eyef,
            }
        )
    return in_maps


_ROW_TGT = None


def _row_targets():
    """_ROW_TGT[g][r_local] = global row offset within the core's 1024 rows."""
    global _ROW_TGT
    if _ROW_TGT is None:
        r = np.arange(1024)
        p_, rem = r // 512, r % 512
        d_, s_l = rem // 8, rem % 8
        _ROW_TGT = [512 * p_ + 8 * d_ + (s_l + 2 * g) % 8 for g in range(4)]
    return _ROW_TGT


def run_spmd(inputs, **kwargs):
    """Run the SPMD kernel; returns (full_output, BassKernelResults)."""
    nc = _get_module()
    in_maps = _prep_core_inputs(inputs)
    res = run_bass_kernel_spmd(nc, in_maps, core_ids=list(range(N_CORES)), **kwargs)
    tgt = _row_targets()
    full = np.empty((B, N, C), np.float32)
    for core in range(N_CORES):
        b, g = divmod(core, 4)
        full[b, 1024 * g + tgt[g], :] = res.results[core]["out"]
    return full, res


def kernel(**inputs) -> np.ndarray:
    full, _ = run_spmd(inputs)
    return full


# revision 11
# speedup vs baseline: 8.0902x; 7.7512x over previous
"""Trainium2 Bass kernel for PVT-style spatial-reduction multi-head attention.

Problem (hardcoded shapes, fp32 inputs):
  x [2, 4096, 512]; Wq [512,512]; Wconv [512,512,2,2] (OIHW, stride 2);
  LayerNorm over the conv's flattened spatial dim (M=1024); Wkv [1024,1024];
  attention with q [B,8,4096,64], k/v [B,8,512,64]; "faithful" reshape
  (out.transpose(0,1,3,2).reshape(B,-1,512)) before Wproj [512,512].

Sharding: 8 cores = (batch b in {0,1}) x (head-pair g in {0..3}).

v3 design vs the v2 baseline (113 us):
 - x is sent host-side in a tap-expanded transposed layout Xp[ic, (m, tap)]
   (n = 128i + 64di + 2j + dj; m = 32i + j; tap = 2di + dj), so no on-chip
   x transpose. Q consumes Xp directly; the resulting within-chunk column
   permutation of q (and of the attention output) is absorbed into a
   host-side row permutation of Wproj.
 - The stride-2 2x2 VALID conv is non-overlapping, computed TRANSPOSED
   (xcT [m, o]) and m-sharded 4 ways across the cores of a batch; the full
   xcT is restored with a DRAM AllGather (bf16, 256KB -> 1MB per core).
   Per-core m-block assignment is encoded by host-side rotation of Xp's
   m-blocks (SPMD program identical across cores); the output-row
   permutation this induces is undone on the host after the run.
 - LayerNorm is folded algebraically: gamma into Wkv rows (host), beta+bkv
   into a bias row s (host), so kv = rs_c*(xcT @ Wkv' - mu_c t + s*sqrtve_c)
   with the mu/s terms as K=1 rank-1 PE matmuls into the same PSUM group.
   The per-position scale rs_c = rsqrt(var_c+eps) folds into the softmax
   exp scale (k side, per-partition AP scale) and the vaug scale (v side).
   Stats (sum x, sum x^2) come from ones-vector PE matmuls + a 4KB DRAM
   AllReduce; rsqrt is computed as exp(-0.5*ln(v)) so the ACT engine stays
   on the one activation table that holds exp/ln/copy/square/identity
   (no 1283ns table reloads).
 - Softmax denominators are 1-column PE matmuls (phat^T @ ones) written
   directly in the transposed [n, ch] layout (no DVE accumulate pass, no
   ones-augmented AV columns).
 - Engine balance: ACT does exp (+ tiny ln/exp stats, kv/proj epilogues),
   gpsimd does the AV PSUM->SBUF copies, DVE does Q epilogue + AV
   normalize + small stats math.
"""

import sys

sys.path.insert(0, "/opt/trn_rl_repo")

import math

import numpy as np
import ml_dtypes

import concourse.bass as bass
import concourse.bacc as bacc
import concourse.mybir as mybir
import concourse.tile as tile
from concourse.bass_utils import run_bass_kernel_spmd

F32 = mybir.dt.float32
BF16 = mybir.dt.bfloat16
NP_BF16 = ml_dtypes.bfloat16

B, N, C = 2, 4096, 512
NH, HD, SR = 8, 64, 2
M = 1024
EPS = 1e-5
N_CORES = 8
GROUPS = [[0, 1, 2, 3], [4, 5, 6, 7]]

# rows16 row map ([8, 512] bf16 host constants)
ROW_BCONV = 0
ROW_BPROJ = 1
ROW_TK = 2
ROW_TV = 3
ROW_SK = 4
ROW_SV = 5
ROW_ONES = 6


def _build_module(reps=1, bench_internal=False):
    nc = bacc.Bacc("TRN2", target_bir_lowering=False, debug=False)

    # ---- per-core DRAM tensors (data differs per core, shapes identical) ----
    KIND = "Internal" if bench_internal else "ExternalInput"
    xp = nc.dram_tensor("xp", [C, N], BF16, kind=KIND).ap()
    wq = nc.dram_tensor("wq", [C, 128], BF16, kind=KIND).ap()
    wconvt = nc.dram_tensor("wconvt", [C, 4, C], BF16, kind=KIND).ap()
    wkv2 = nc.dram_tensor("wkv2", [M, 256], BF16, kind=KIND).ap()
    wproj = nc.dram_tensor("wproj", [C, C], BF16, kind=KIND).ap()
    small = nc.dram_tensor("small", [128, 2], F32, kind="ExternalInput").ap()
    rows16 = nc.dram_tensor("rows16", [1, 8 * C], BF16, kind=KIND).ap()
    eye128 = nc.dram_tensor("eye128", [128, 128], BF16, kind=KIND).ap()
    eyef = nc.dram_tensor("eyef", [8, 8], F32, kind=KIND).ap()
    out = nc.dram_tensor("out", [2 * C, C], F32, kind="ExternalOutput").ap()

    AX = mybir.AxisListType.X
    OP = mybir.AluOpType
    AF = mybir.ActivationFunctionType
    LN8 = math.log(0.125)

    with tile.TileContext(nc) as tc:
        import contextlib

        with contextlib.ExitStack() as ctx:
            persist = ctx.enter_context(tc.tile_pool(name="persist", bufs=1))
            stage = ctx.enter_context(tc.tile_pool(name="stage", bufs=3))
            ps512 = ctx.enter_context(tc.tile_pool(name="ps512", bufs=3, space="PSUM"))
            psav = ctx.enter_context(tc.tile_pool(name="psav", bufs=2, space="PSUM"))
            psden = ctx.enter_context(tc.tile_pool(name="psden", bufs=1, space="PSUM"))
            pstp = ctx.enter_context(tc.tile_pool(name="pstp", bufs=2, space="PSUM"))

            for _rep in range(reps):
                # ---------------- weight / const loads ----------------
                wq_sb = []
                for k in range(4):
                    t = persist.tile([128, 128], BF16, name=f"wq_sb{k}", tag=f"wq{k}")
                    nc.sync.dma_start(t[:], wq[128 * k : 128 * (k + 1), :])
                    wq_sb.append(t)

                wconv_sb = []  # [ic_t] -> [128 ic, (tap 4, o 512)]
                for kt in range(4):
                    t = persist.tile([128, 2048], BF16, name=f"wconv{kt}", tag=f"wc{kt}")
                    nc.sync.dma_start(t[:], wconvt[128 * kt : 128 * (kt + 1), :, :])
                    wconv_sb.append(t)

                wkv_sb = []  # [mt] -> [128 m, 256] (cols 0:128 k, 128:256 v)
                for k in range(8):
                    t = persist.tile([128, 256], BF16, name=f"wkv_sb{k}", tag=f"wkv{k}")
                    nc.sync.dma_start(t[:], wkv2[128 * k : 128 * (k + 1), :])
                    wkv_sb.append(t)

                wproj_sb = []
                for ct in range(4):
                    t = persist.tile([128, 512], BF16, name=f"wproj{ct}", tag=f"wp{ct}")
                    nc.sync.dma_start(t[:], wproj[128 * ct : 128 * (ct + 1), :])
                    wproj_sb.append(t)

                small_sb = persist.tile([128, 2], F32, name="small_sb", tag="small")
                nc.sync.dma_start(small_sb[:], small[:, :])
                rows_sb = persist.tile([1, 4096], BF16, name="rows_sb", tag="rows16")
                nc.sync.dma_start(rows_sb[:], rows16[:, :])

                def crow(r, n=512):
                    return rows_sb[0:1, 512 * r : 512 * r + n]
                eye_sb = persist.tile([128, 128], BF16, name="eye_sb", tag="eye128")
                nc.sync.dma_start(eye_sb[:], eye128[:, :])
                eyef_sb = persist.tile([8, 8], F32, name="eyef_sb", tag="eyef")
                nc.sync.dma_start(eyef_sb[:], eyef[:, :])
                onec_sb = persist.tile([128, 1], BF16, name="onec_sb", tag="onec")
                nc.vector.memset(onec_sb[:], 1.0)
                ln8_sb = persist.tile([128, 1], F32, name="ln8_sb", tag="ln8")
                nc.vector.memset(ln8_sb[:], LN8)

                bq_col = small_sb[:, 0:1]

                # ---------------- x load (conv slices first) ----------------
                xp_sb = []  # [ic_t] -> [128 ic, 4096 (m,tap)]
                for kt in range(4):
                    t = persist.tile([128, N], BF16, name=f"xp_sb{kt}", tag=f"xp{kt}")
                    xp_sb.append(t)
                for kt in range(4):
                    nc.sync.dma_start(
                        xp_sb[kt][:, 0:1024], xp[128 * kt : 128 * (kt + 1), 0:1024]
                    )
                for kt in range(4):
                    nc.sync.dma_start(
                        xp_sb[kt][:, 1024:4096], xp[128 * kt : 128 * (kt + 1), 1024:4096]
                    )
                xp4 = [t.rearrange("p (m tap) -> p m tap", m=M, tap=4) for t in xp_sb]

                # ---------------- conv (all 8 m-blocks), stats ----------------
                xcl_sb = []  # conv out [128 m, 512 o] bf16 per m-block
                sq_sb = []
                for l in range(8):
                    c_ps = ps512.tile([128, 512], F32, name="c_ps", tag="mm512")
                    first = True
                    for kt in range(4):
                        for tap in range(4):
                            nc.tensor.matmul(
                                c_ps[:],
                                xp4[kt][:, 128 * l : 128 * (l + 1), tap],
                                wconv_sb[kt][:, 512 * tap : 512 * (tap + 1)],
                                start=first,
                                stop=False,
                            )
                            first = False
                    # += ones (x) bconv  (rank-1 bias over m partitions)
                    nc.tensor.matmul(
                        c_ps[:], crow(ROW_ONES, 128), crow(ROW_BCONV),
                        start=False, stop=True,
                    )
                    xcl = persist.tile([128, 512], BF16, name=f"xcl{l}", tag=f"xcl{l}")
                    nc.scalar.activation(xcl[:], c_ps[:], AF.Copy)
                    sq = persist.tile([128, 512], BF16, name=f"sq{l}", tag=f"sql{l}")
                    nc.vector.tensor_mul(sq[:], xcl[:], xcl[:])
                    xcl_sb.append(xcl)
                    sq_sb.append(sq)
                sx_ps = ps512.tile([1, 512], F32, name="sx_ps", tag="mm512")
                sq_ps = ps512.tile([1, 512], F32, name="sq_ps", tag="mm512")
                for l in range(8):
                    nc.tensor.matmul(
                        sx_ps[:], onec_sb[:], xcl_sb[l][:],
                        start=(l == 0), stop=(l == 7),
                    )
                    nc.tensor.matmul(
                        sq_ps[:], onec_sb[:], sq_sb[l][:],
                        start=(l == 0), stop=(l == 7),
                    )
                srow_sb = persist.tile([1, 1024], F32, name="srow_sb", tag="srow")
                nc.vector.tensor_copy(srow_sb[0:1, 0:512], sx_ps[:])
                nc.vector.tensor_copy(srow_sb[0:1, 512:1024], sq_ps[:])
                sx_row = srow_sb[0:1, 0:512]
                sq_row = srow_sb[0:1, 512:1024]

                # ---------------- Q projection ----------------
                qt_sb = persist.tile([128, N], BF16, name="qt_sb", tag="qt")
                for ch in range(8):
                    q_ps = ps512.tile([128, 512], F32, name="q_ps", tag="mm512")
                    for k in range(4):
                        nc.tensor.matmul(
                            q_ps[:],
                            wq_sb[k][:],
                            xp4[k][:, 128 * ch : 128 * (ch + 1), :],
                            start=(k == 0),
                            stop=(k == 3),
                        )
                    nc.vector.tensor_scalar_add(
                        qt_sb[:, 512 * ch : 512 * (ch + 1)], q_ps[:], bq_col
                    )

                # ---------------- stats math ----------------
                murow = stage.tile([1, 512], F32, name="murow", tag="murow", bufs=1)
                nc.vector.tensor_scalar_mul(murow[:], sx_row, 1.0 / M)
                negmu16 = persist.tile([1, 512], BF16, name="negmu16", tag="negmu")
                nc.vector.tensor_scalar_mul(negmu16[:], sx_row, -1.0 / M)
                verow = persist.tile([1, 512], F32, name="verow", tag="verow")
                nc.vector.tensor_scalar(
                    out=verow[:], in0=sq_row,
                    scalar1=1.0 / M, scalar2=EPS, op0=OP.mult, op1=OP.add,
                )
                mu2 = stage.tile([1, 512], F32, name="mu2", tag="mu2", bufs=1)
                nc.vector.tensor_mul(mu2[:], murow[:], murow[:])
                nc.vector.tensor_sub(verow[:], verow[:], mu2[:])
                # sqrtve row (bf16) = exp(0.5 ln ve)
                lrow = stage.tile([1, 512], F32, name="lrow", tag="lrow", bufs=1)
                nc.scalar.activation(lrow[:], verow[:], AF.Ln)
                sqve16 = persist.tile([1, 512], BF16, name="sqve16", tag="sqve")
                nc.scalar.activation(sqve16[:], lrow[:], AF.Exp, scale=0.5)
                # columns: ve -> [128, 4] via PE transpose, then exp/ln scales
                vecol_ps = ps512.tile([128, 4], F32, name="vecol_ps", tag="mm512")
                for j in range(4):
                    nc.tensor.transpose(
                        vecol_ps[:, j : j + 1],
                        verow[:, 128 * j : 128 * (j + 1)],
                        eyef_sb[0:1, 0:1],
                    )
                lcol = stage.tile([128, 4], F32, name="lcol", tag="lcol", bufs=1)
                nc.scalar.activation(lcol[:], vecol_ps[:], AF.Ln)
                esc_col = persist.tile([128, 4], F32, name="esc_col", tag="esc")
                nc.scalar.activation(esc_col[:], lcol[:], AF.Exp, scale=-0.5, bias=ln8_sb[:])
                vsc_col = persist.tile([128, 4], F32, name="vsc_col", tag="vsc")
                nc.scalar.activation(vsc_col[:], lcol[:], AF.Exp, scale=-0.5)

                # ---------------- KV ----------------
                kT_sb = persist.tile([128, 512], BF16, name="kT_sb", tag="kT")
                vT_sb = persist.tile([128, 512], BF16, name="vT_sb", tag="vT")
                for which, lo, t_row, s_row, dst in (
                    ("k", 0, ROW_TK, ROW_SK, kT_sb),
                    ("v", 128, ROW_TV, ROW_SV, vT_sb),
                ):
                    kv_ps = ps512.tile([128, 512], F32, name="kv_ps", tag="mm512")
                    for k in range(8):
                        nc.tensor.matmul(
                            kv_ps[:], wkv_sb[k][:, lo : lo + 128], xcl_sb[k][:],
                            start=(k == 0), stop=False,
                        )
                    nc.tensor.matmul(
                        kv_ps[:], crow(t_row, 128), negmu16[:],
                        start=False, stop=False,
                    )
                    nc.tensor.matmul(
                        kv_ps[:], crow(s_row, 128), sqve16[:],
                        start=False, stop=True,
                    )
                    nc.scalar.activation(dst[:], kv_ps[:], AF.Copy)

                # vaug[p][mt]: [128 c, 64 d] bf16, v^T with rs folded
                vaug_sb = []
                for p in range(2):
                    row = []
                    for mt in range(4):
                        t = persist.tile(
                            [128, 64], BF16, name=f"vaug{p}_{mt}", tag=f"va{p}{mt}"
                        )
                        tp = pstp.tile([128, 64], BF16, name="tpv", tag="ptp")
                        nc.tensor.transpose(
                            tp[:],
                            vT_sb[64 * p : 64 * (p + 1), 128 * mt : 128 * (mt + 1)],
                            eye_sb[64 * p : 64 * (p + 1), 64 * p : 64 * (p + 1)],
                        )
                        nc.vector.tensor_scalar_mul(
                            t[:], tp[:], vsc_col[:, mt : mt + 1]
                        )
                        row.append(t)
                    vaug_sb.append(row)

                # ---------------- attention ----------------
                avT_sb = [
                    persist.tile([64, N], BF16, name=f"avT{p}", tag=f"avT{p}")
                    for p in range(2)
                ]
                den_all = psden.tile([128, 64], F32, name="den_all", tag="den")
                den_ps = [den_all[:, 32 * p : 32 * (p + 1)] for p in range(2)]
                den4 = den_all.rearrange("p (g ct c) -> p g ct c", g=2, ct=4, c=8)
                out2dT = []
                for p in range(2):
                    o_row = []
                    for ct in range(4):
                        t = persist.tile(
                            [128, 512], BF16, name=f"o2dT{p}_{ct}", tag=f"o2{p}{ct}"
                        )
                        o_row.append(t)
                    out2dT.append(o_row)

                def norm_stage(ch):
                    # recip + AV transpose + normalize for a finished chunk
                    for p in range(2):
                        rc = stage.tile([128, 4], F32, name="rc", tag="rc", bufs=4)
                        nc.vector.reciprocal(rc[:], den4[:, p, :, ch])
                        for ct in range(4):
                            i = 4 * ch + ct
                            tp = pstp.tile([128, 64], BF16, name="tpav", tag="ptp")
                            nc.tensor.transpose(
                                tp[:],
                                avT_sb[p][:, 128 * i : 128 * (i + 1)],
                                eye_sb[0:64, 0:64],
                            )
                            o3 = out2dT[p][ct].rearrange(
                                "p (d s) -> p d s", d=64, s=8
                            )
                            nc.vector.tensor_scalar_mul(
                                o3[:, :, ch], tp[:], rc[:, ct : ct + 1]
                            )

                for ch in range(8):
                    phat = {0: [], 1: []}
                    for mt in range(4):
                        for p in range(2):
                            s_ps = ps512.tile([128, 512], F32, name="s_ps", tag="mm512")
                            nc.tensor.matmul(
                                s_ps[:],
                                kT_sb[64 * p : 64 * (p + 1), 128 * mt : 128 * (mt + 1)],
                                qt_sb[64 * p : 64 * (p + 1), 512 * ch : 512 * (ch + 1)],
                                start=True,
                                stop=True,
                            )
                            ph = stage.tile(
                                [128, 512], BF16, name="phat", tag="phat", bufs=8
                            )
                            nc.scalar.activation(
                                ph[:], s_ps[:], AF.Exp, scale=esc_col[:, mt : mt + 1]
                            )
                            phat[p].append(ph)
                    for p in range(2):
                        av_ps = psav.tile([64, 512], F32, name="av_ps", tag="psav")
                        for mt in range(4):
                            nc.tensor.matmul(
                                av_ps[:],
                                vaug_sb[p][mt][:],
                                phat[p][mt][:],
                                start=(mt == 0),
                                stop=(mt == 3),
                            )
                        nc.vector.tensor_copy(
                            avT_sb[p][:, 512 * ch : 512 * (ch + 1)], av_ps[:]
                        )
                        # denominators, directly transposed: den[n, ch]
                        for ct in range(4):
                            for mt in range(4):
                                nc.tensor.matmul(
                                    den_ps[p][:, 8 * ct + ch : 8 * ct + ch + 1],
                                    phat[p][mt][:, 128 * ct : 128 * (ct + 1)],
                                    onec_sb[:],
                                    start=(mt == 0),
                                    stop=(mt == 3),
                                )
                    if ch > 0:
                        norm_stage(ch - 1)
                norm_stage(7)

                # ---------------- projection + output ----------------
                for p in range(2):
                    for rt in range(4):
                        pr_ps = ps512.tile([128, 512], F32, name="pr_ps", tag="mm512")
                        for ct in range(4):
                            nc.tensor.matmul(
                                pr_ps[:],
                                out2dT[p][ct][:, 128 * rt : 128 * (rt + 1)],
                                wproj_sb[ct][:],
                                start=(ct == 0),
                                stop=False,
                            )
                        nc.tensor.matmul(
                            pr_ps[:], crow(ROW_ONES, 128), crow(ROW_BPROJ),
                            start=False, stop=True,
                        )
                        of = stage.tile([128, 512], F32, name="of", tag="of", bufs=3)
                        nc.scalar.activation(of[:], pr_ps[:], AF.Copy)
                        r0 = 512 * p + 128 * rt
                        nc.sync.dma_start(out[r0 : r0 + 128, :], of[:])

    nc.compile()
    return nc


_NC_CACHE = None


def _get_module():
    global _NC_CACHE
    if _NC_CACHE is None:
        _NC_CACHE = _build_module()
    return _NC_CACHE


def _prep_core_inputs(inputs):
    """Host-side sharding: layout/permute/cast weights, build 8 in_maps."""
    x = np.asarray(inputs["x"], np.float32)
    Wq = np.asarray(inputs["Wq"], np.float32)
    bq = np.asarray(inputs["bq"], np.float32)
    Wconv = np.asarray(inputs["Wconv"], np.float32)
    bconv = np.asarray(inputs["bconv"], np.float32)
    gamma = np.asarray(inputs["gamma"], np.float32)
    beta = np.asarray(inputs["beta"], np.float32)
    Wkv = np.asarray(inputs["Wkv"], np.float32)
    bkv = np.asarray(inputs["bkv"], np.float32)
    Wproj = np.asarray(inputs["Wproj"], np.float32)
    bproj = np.asarray(inputs["bproj"], np.float32)

    # Xp: [ic, (m, tap)]; n = 128i + 64di + 2j + dj, m = 32i+j, tap = 2di+dj
    xp_g = []
    for b in range(B):
        xt = x[b].T.reshape(C, 32, 2, 32, 2)  # [ic, i, di, j, dj]
        xt = np.ascontiguousarray(
            xt.transpose(0, 1, 3, 2, 4).reshape(C, 8, 512)  # [ic, blk, rest]
        )
        xp_g.append(xt)

    wconvt = np.ascontiguousarray(
        Wconv.transpose(1, 2, 3, 0).reshape(C, 4, C)
    ).astype(NP_BF16)

    wkvp = gamma[:, None] * Wkv
    s_full = beta @ Wkv + bkv

    # Wproj row permutation: u' = 128 i' + 4 j + 2 di + dj -> n' = 128 i' + 64 di + 2 j + dj
    up = np.arange(C)
    i_, j_ = up // 128, (up % 128) // 4
    di, dj = (up % 4) // 2, up % 2
    nprime = 128 * i_ + 64 * di + 2 * j_ + dj
    wproj_perm = np.ascontiguousarray(Wproj[nprime, :]).astype(NP_BF16)

    eye128 = np.eye(128, dtype=np.float32).astype(NP_BF16)
    eyef = np.eye(8, dtype=np.float32)

    in_maps = []
    for core in range(N_CORES):
        b, g = divmod(core, 4)
        kcols = slice(128 * g, 128 * (g + 1))
        vcols = slice(512 + 128 * g, 512 + 128 * (g + 1))
        xp_loc = np.ascontiguousarray(xp_g[b].reshape(C, N)).astype(NP_BF16)
        small = np.zeros((128, 2), np.float32)
        small[:, 0] = bq[kcols]
        rows16 = np.zeros((8, C), np.float32)
        rows16[ROW_BCONV] = bconv
        rows16[ROW_BPROJ] = bproj
        rows16[ROW_TK, 0:128] = wkvp[:, kcols].sum(0)
        rows16[ROW_TV, 0:128] = wkvp[:, vcols].sum(0)
        rows16[ROW_SK, 0:128] = s_full[kcols]
        rows16[ROW_SV, 0:128] = s_full[vcols]
        rows16[ROW_ONES] = 1.0
        wkv2 = np.concatenate([wkvp[:, kcols], wkvp[:, vcols]], axis=1)
        in_maps.append(
            {
                "xp": xp_loc,
                "wq": np.ascontiguousarray(Wq[:, kcols]).astype(NP_BF16),
                "wconvt": wconvt,
                "wkv2": np.ascontiguousarray(wkv2).astype(NP_BF16),
                "wproj": wproj_perm,
                "small": small,
                "rows16": rows16.reshape(1, 8 * C).astype(NP_BF16),
                "eye128": eye128,
                "eyef": eyef,
            }
        )
    return in_maps


_ROW_TGT = None


def _row_targets():
    """_ROW_TGT[g][r_local] = global row offset within the core's 1024 rows."""
    global _ROW_TGT
    if _ROW_TGT is None:
        r = np.arange(1024)
        p_, rem = r // 512, r % 512
        d_, s_l = rem // 8, rem % 8
        _ROW_TGT = [512 * p_ + 8 * d_ + (s_l + 2 * g) % 8 for g in range(4)]
    return _ROW_TGT


def run_spmd(inputs, **kwargs):
    """Run the SPMD kernel; returns (full_output, BassKernelResults)."""
    nc = _get_module()
    in_maps = _prep_core_inputs(inputs)
    res = run_bass_kernel_spmd(nc, in_maps, core_ids=list(range(N_CORES)), **kwargs)
    full = np.empty((B, N, C), np.float32)
    for core in range(N_CORES):
        b, g = divmod(core, 4)
        full[b, 1024 * g : 1024 * (g + 1), :] = res.results[core]["out"]
    return full, res


def kernel(**inputs) -> np.ndarray:
    full, _ = run_spmd(inputs)
    return full


# revision 14
# speedup vs baseline: 63.4078x; 7.8376x over previous
"""Trainium2 Bass kernel for PVT-style spatial-reduction multi-head attention.

Problem (hardcoded shapes, fp32 inputs):
  x [2, 4096, 512]; Wq [512,512]; Wconv [512,512,2,2] (OIHW, stride 2);
  LayerNorm over the conv's flattened spatial dim (M=1024); Wkv [1024,1024];
  attention with q [B,8,4096,64], k/v [B,8,512,64]; "faithful" reshape
  (out.transpose(0,1,3,2).reshape(B,-1,512)) before Wproj [512,512].

Sharding: 8 cores = (batch b in {0,1}) x (head-pair g in {0..3}).
Core (b,g) computes heads {2g, 2g+1} of batch b and writes output rows
[b, 1024g : 1024g+1024, :].

v3 design vs the v2 baseline (113 us):
 - x is sent host-side in a tap-expanded transposed layout Xp[ic, (m, tap)]
   (n = 128i + 64di + 2j + dj; m = 32i + j; tap = 2di + dj), so there is no
   on-chip x transpose. Q and the conv consume Xp directly; the resulting
   within-chunk column permutation of q (and of the attention output) is
   absorbed into a host-side row permutation of Wproj.
 - The stride-2 2x2 VALID conv is non-overlapping and is computed
   TRANSPOSED (xcT [m, o] = Xp^T @ Wconv'), which feeds the KV matmul with
   no transpose stage. (A 4-way m-sharded variant with a DRAM AllGather was
   tried and reverted: one collective_compute costs ~150-250 us through
   this NRT path, dwarfing the 20 us of saved conv work.)
 - LayerNorm is folded algebraically: gamma into Wkv rows (host), beta+bkv
   into a bias row s (host), so kv_noscale = xcT @ Wkv' - mu_c t + s*sqrtve_c
   with the mu/s terms as K=1 rank-1 PE matmuls into the same PSUM group.
   The per-position scale rs_c = rsqrt(var_c+eps) folds into the softmax
   exp scale (k side, per-partition AP scale) and the vaug scale (v side).
   Stats (sum x, sum x^2) come from ones-vector PE matmuls; rsqrt is
   computed as exp(-0.5*ln(v)) so the ACT engine stays on the single
   activation table holding exp/ln/copy/square/identity (no 1283ns act
   table reloads anywhere in the kernel).
 - AV is computed n-partitioned (av2[n, d] = sum_c phat[c,n]^T [v^T*rs|1]),
   so there is no AV transpose stage, and softmax denominators ride along
   as a 65th rhs column, landing pre-transposed for the normalize.
 - Engine balance: ACT does exp + PSUM->SBUF copies (one act table), DVE
   does Q epilogue, squares, AV normalize; reciprocal+normalize are
   interleaved into the chunk loop so only the projection remains as tail.
"""

import sys

sys.path.insert(0, "/opt/trn_rl_repo")

import math

import numpy as np
import ml_dtypes

import concourse.bass as bass
import concourse.bacc as bacc
import concourse.mybir as mybir
import concourse.tile as tile
from concourse.bass_utils import run_bass_kernel_spmd

F32 = mybir.dt.float32
BF16 = mybir.dt.bfloat16
NP_BF16 = ml_dtypes.bfloat16

B, N, C = 2, 4096, 512
NH, HD, SR = 8, 64, 2
M = 1024
EPS = 1e-5
N_CORES = 8

# rows16 row map ([8, 512] bf16 host constants)
ROW_BCONV = 0
ROW_BPROJ = 1
ROW_TK = 2
ROW_TV = 3
ROW_SK = 4
ROW_SV = 5
ROW_ONES = 6


def _build_module(reps=1, bench_internal=False):
    nc = bacc.Bacc("TRN2", target_bir_lowering=False, debug=False)

    # ---- per-core DRAM tensors (data differs per core, shapes identical) ----
    KIND = "Internal" if bench_internal else "ExternalInput"
    xp = nc.dram_tensor("xp", [C, N], BF16, kind=KIND).ap()
    wq = nc.dram_tensor("wq", [C, 128], BF16, kind=KIND).ap()
    wconvt = nc.dram_tensor("wconvt", [C, 4, C], BF16, kind=KIND).ap()
    wkv2 = nc.dram_tensor("wkv2", [M, 256], BF16, kind=KIND).ap()
    wproj = nc.dram_tensor("wproj", [C, C], BF16, kind=KIND).ap()
    small = nc.dram_tensor("small", [128, 2], F32, kind="ExternalInput").ap()
    rows16 = nc.dram_tensor("rows16", [1, 8 * C], BF16, kind=KIND).ap()
    eye128 = nc.dram_tensor("eye128", [128, 128], BF16, kind=KIND).ap()
    eyef = nc.dram_tensor("eyef", [8, 8], F32, kind=KIND).ap()
    OKIND = "Internal" if bench_internal else "ExternalOutput"
    out = nc.dram_tensor("out", [2 * C, C], F32, kind=OKIND).ap()
    dummy = (
        nc.dram_tensor("bench_out", [1, 2], F32, kind="ExternalOutput").ap()
        if bench_internal
        else None
    )

    AX = mybir.AxisListType.X
    OP = mybir.AluOpType
    AF = mybir.ActivationFunctionType
    LN8 = math.log(0.125)

    with tile.TileContext(nc) as tc:
        import contextlib

        with contextlib.ExitStack() as ctx:
            persist = ctx.enter_context(tc.tile_pool(name="persist", bufs=1))
            stage = ctx.enter_context(tc.tile_pool(name="stage", bufs=3))
            ps512 = ctx.enter_context(tc.tile_pool(name="ps512", bufs=3, space="PSUM"))
            psa2 = ctx.enter_context(tc.tile_pool(name="psa2", bufs=3, space="PSUM"))
            pstp = ctx.enter_context(tc.tile_pool(name="pstp", bufs=2, space="PSUM"))

            for _rep in range(reps):
                # ---------------- weight / const loads ----------------
                wq_sb = []
                for k in range(4):
                    t = persist.tile([128, 128], BF16, name=f"wq_sb{k}", tag=f"wq{k}")
                    nc.sync.dma_start(t[:], wq[128 * k : 128 * (k + 1), :])
                    wq_sb.append(t)

                wconv_sb = []  # [ic_t] -> [128 ic, (tap 4, o 512)]
                for kt in range(4):
                    t = persist.tile([128, 2048], BF16, name=f"wconv{kt}", tag=f"wc{kt}")
                    nc.sync.dma_start(t[:], wconvt[128 * kt : 128 * (kt + 1), :, :])
                    wconv_sb.append(t)

                wkv_sb = []  # [mt] -> [128 m, 256] (cols 0:128 k, 128:256 v)
                for k in range(8):
                    t = persist.tile([128, 256], BF16, name=f"wkv_sb{k}", tag=f"wkv{k}")
                    nc.sync.dma_start(t[:], wkv2[128 * k : 128 * (k + 1), :])
                    wkv_sb.append(t)

                wproj_sb = []
                for ct in range(4):
                    t = persist.tile([128, 512], BF16, name=f"wproj{ct}", tag=f"wp{ct}")
                    nc.sync.dma_start(t[:], wproj[128 * ct : 128 * (ct + 1), :])
                    wproj_sb.append(t)

                small_sb = persist.tile([128, 2], F32, name="small_sb", tag="small")
                nc.sync.dma_start(small_sb[:], small[:, :])
                if dummy is not None and _rep == 0:
                    nc.sync.dma_start(dummy[:, :], small[0:1, 0:2])
                rows_sb = persist.tile([1, 4096], BF16, name="rows_sb", tag="rows16")
                nc.sync.dma_start(rows_sb[:], rows16[:, :])

                def crow(r, n=512):
                    return rows_sb[0:1, 512 * r : 512 * r + n]
                eye_sb = persist.tile([128, 128], BF16, name="eye_sb", tag="eye128")
                nc.sync.dma_start(eye_sb[:], eye128[:, :])
                eyef_sb = persist.tile([8, 8], F32, name="eyef_sb", tag="eyef")
                nc.sync.dma_start(eyef_sb[:], eyef[:, :])
                onec_sb = persist.tile([128, 1], BF16, name="onec_sb", tag="onec")
                nc.vector.memset(onec_sb[:], 1.0)
                ln8_sb = persist.tile([128, 1], F32, name="ln8_sb", tag="ln8")
                nc.vector.memset(ln8_sb[:], LN8)

                bq_col = small_sb[:, 0:1]

                # ---------------- x load (conv slices first) ----------------
                xp_sb = []  # [ic_t] -> [128 ic, 4096 (m,tap)]
                for kt in range(4):
                    t = persist.tile([128, N], BF16, name=f"xp_sb{kt}", tag=f"xp{kt}")
                    xp_sb.append(t)
                for kt in range(4):
                    nc.sync.dma_start(
                        xp_sb[kt][:, 0:1024], xp[128 * kt : 128 * (kt + 1), 0:1024]
                    )
                for kt in range(4):
                    nc.sync.dma_start(
                        xp_sb[kt][:, 1024:4096], xp[128 * kt : 128 * (kt + 1), 1024:4096]
                    )
                xp4 = [t.rearrange("p (m tap) -> p m tap", m=M, tap=4) for t in xp_sb]

                # ---------------- conv (all 8 m-blocks), stats ----------------
                xcl_sb = []  # conv out [128 m, 512 o] bf16 per m-block
                sq_sb = []
                for l in range(8):
                    c_ps = ps512.tile([128, 512], F32, name="c_ps", tag="mm512")
                    first = True
                    for kt in range(4):
                        for tap in range(4):
                            nc.tensor.matmul(
                                c_ps[:],
                                xp4[kt][:, 128 * l : 128 * (l + 1), tap],
                                wconv_sb[kt][:, 512 * tap : 512 * (tap + 1)],
                                start=first,
                                stop=False,
                            )
                            first = False
                    # += ones (x) bconv  (rank-1 bias over m partitions)
                    nc.tensor.matmul(
                        c_ps[:], crow(ROW_ONES, 128), crow(ROW_BCONV),
                        start=False, stop=True,
                    )
                    xcl = persist.tile([128, 512], BF16, name=f"xcl{l}", tag=f"xcl{l}")
                    nc.scalar.activation(xcl[:], c_ps[:], AF.Copy)
                    sq = persist.tile([128, 512], BF16, name=f"sq{l}", tag=f"sql{l}")
                    nc.vector.tensor_mul(sq[:], xcl[:], xcl[:])
                    xcl_sb.append(xcl)
                    sq_sb.append(sq)
                sx_ps = ps512.tile([1, 512], F32, name="sx_ps", tag="mm512")
                sq_ps = ps512.tile([1, 512], F32, name="sq_ps", tag="mm512")
                for l in range(8):
                    nc.tensor.matmul(
                        sx_ps[:], onec_sb[:], xcl_sb[l][:],
                        start=(l == 0), stop=(l == 7),
                    )
                    nc.tensor.matmul(
                        sq_ps[:], onec_sb[:], sq_sb[l][:],
                        start=(l == 0), stop=(l == 7),
                    )
                srow_sb = persist.tile([1, 1024], F32, name="srow_sb", tag="srow")
                nc.vector.tensor_copy(srow_sb[0:1, 0:512], sx_ps[:])
                nc.vector.tensor_copy(srow_sb[0:1, 512:1024], sq_ps[:])
                sx_row = srow_sb[0:1, 0:512]
                sq_row = srow_sb[0:1, 512:1024]

                # ---------------- Q projection ----------------
                qt_sb = persist.tile([128, N], BF16, name="qt_sb", tag="qt")
                for ch in range(8):
                    q_ps = ps512.tile([128, 512], F32, name="q_ps", tag="mm512")
                    for k in range(4):
                        nc.tensor.matmul(
                            q_ps[:],
                            wq_sb[k][:],
                            xp4[k][:, 128 * ch : 128 * (ch + 1), :],
                            start=(k == 0),
                            stop=(k == 3),
                        )
                    nc.vector.tensor_scalar_add(
                        qt_sb[:, 512 * ch : 512 * (ch + 1)], q_ps[:], bq_col
                    )

                # ---------------- stats math ----------------
                murow = stage.tile([1, 512], F32, name="murow", tag="murow", bufs=1)
                nc.vector.tensor_scalar_mul(murow[:], sx_row, 1.0 / M)
                negmu16 = persist.tile([1, 512], BF16, name="negmu16", tag="negmu")
                nc.vector.tensor_scalar_mul(negmu16[:], sx_row, -1.0 / M)
                verow = persist.tile([1, 512], F32, name="verow", tag="verow")
                nc.vector.tensor_scalar(
                    out=verow[:], in0=sq_row,
                    scalar1=1.0 / M, scalar2=EPS, op0=OP.mult, op1=OP.add,
                )
                mu2 = stage.tile([1, 512], F32, name="mu2", tag="mu2", bufs=1)
                nc.vector.tensor_mul(mu2[:], murow[:], murow[:])
                nc.vector.tensor_sub(verow[:], verow[:], mu2[:])
                # sqrtve row (bf16) = exp(0.5 ln ve)
                lrow = stage.tile([1, 512], F32, name="lrow", tag="lrow", bufs=1)
                nc.scalar.activation(lrow[:], verow[:], AF.Ln)
                sqve16 = persist.tile([1, 512], BF16, name="sqve16", tag="sqve")
                nc.scalar.activation(sqve16[:], lrow[:], AF.Exp, scale=0.5)
                # columns: ve -> [128, 4] via PE transpose, then exp/ln scales
                vecol_ps = ps512.tile([128, 4], F32, name="vecol_ps", tag="mm512")
                for j in range(4):
                    nc.tensor.transpose(
                        vecol_ps[:, j : j + 1],
                        verow[:, 128 * j : 128 * (j + 1)],
                        eyef_sb[0:1, 0:1],
                    )
                lcol = stage.tile([128, 4], F32, name="lcol", tag="lcol", bufs=1)
                nc.scalar.activation(lcol[:], vecol_ps[:], AF.Ln)
                esc_col = persist.tile([128, 4], F32, name="esc_col", tag="esc")
                nc.scalar.activation(esc_col[:], lcol[:], AF.Exp, scale=-0.5, bias=ln8_sb[:])
                vsc_col = persist.tile([128, 4], F32, name="vsc_col", tag="vsc")
                nc.scalar.activation(vsc_col[:], lcol[:], AF.Exp, scale=-0.5)

                # ---------------- KV ----------------
                kT_sb = persist.tile([128, 512], BF16, name="kT_sb", tag="kT")
                vT_sb = persist.tile([128, 512], BF16, name="vT_sb", tag="vT")
                for which, lo, t_row, s_row, dst in (
                    ("k", 0, ROW_TK, ROW_SK, kT_sb),
                    ("v", 128, ROW_TV, ROW_SV, vT_sb),
                ):
                    kv_ps = ps512.tile([128, 512], F32, name="kv_ps", tag="mm512")
                    for k in range(8):
                        nc.tensor.matmul(
                            kv_ps[:], wkv_sb[k][:, lo : lo + 128], xcl_sb[k][:],
                            start=(k == 0), stop=False,
                        )
                    nc.tensor.matmul(
                        kv_ps[:], crow(t_row, 128), negmu16[:],
                        start=False, stop=False,
                    )
                    nc.tensor.matmul(
                        kv_ps[:], crow(s_row, 128), sqve16[:],
                        start=False, stop=True,
                    )
                    nc.scalar.activation(dst[:], kv_ps[:], AF.Copy)

                # vaug[p][mt]: [128 c, 64 d] bf16, v^T with rs folded
                vaug_sb = []
                for p in range(2):
                    row = []
                    for mt in range(4):
                        t = persist.tile(
                            [128, 65], BF16, name=f"vaug{p}_{mt}", tag=f"va{p}{mt}"
                        )
                        tp = pstp.tile([128, 64], BF16, name="tpv", tag="ptp")
                        nc.tensor.transpose(
                            tp[:],
                            vT_sb[64 * p : 64 * (p + 1), 128 * mt : 128 * (mt + 1)],
                            eye_sb[64 * p : 64 * (p + 1), 64 * p : 64 * (p + 1)],
                        )
                        nc.vector.tensor_scalar_mul(
                            t[:, 0:64], tp[:], vsc_col[:, mt : mt + 1]
                        )
                        nc.vector.memset(t[:, 64:65], 1.0)
                        row.append(t)
                    vaug_sb.append(row)

                # ---------------- attention ----------------
                # AV is computed n-partitioned: av2[n, (d | den)] = sum_c
                # phat[c, n]^T @ [v^T*rs | 1], so no AV transpose stage and
                # denominators ride along as column 64 of each ct block.
                out2dT = []
                for p in range(2):
                    o_row = []
                    for ct in range(4):
                        t = persist.tile(
                            [128, 512], BF16, name=f"o2dT{p}_{ct}", tag=f"o2{p}{ct}"
                        )
                        o_row.append(t)
                    out2dT.append(o_row)

                for ch in range(8):
                    phat = {0: [], 1: []}
                    for mt in range(4):
                        for p in range(2):
                            s_ps = ps512.tile([128, 512], F32, name="s_ps", tag="mm512")
                            nc.tensor.matmul(
                                s_ps[:],
                                kT_sb[64 * p : 64 * (p + 1), 128 * mt : 128 * (mt + 1)],
                                qt_sb[64 * p : 64 * (p + 1), 512 * ch : 512 * (ch + 1)],
                                start=True,
                                stop=True,
                            )
                            ph = stage.tile(
                                [128, 512], BF16, name="phat", tag="phat", bufs=8
                            )
                            nc.scalar.activation(
                                ph[:], s_ps[:], AF.Exp, scale=esc_col[:, mt : mt + 1]
                            )
                            phat[p].append(ph)
                    for p in range(2):
                        av2 = psa2.tile([128, 260], F32, name="av2", tag="psa2")
                        av3 = av2.rearrange("p (ct u) -> p ct u", ct=4, u=65)
                        for ct in range(4):
                            for mt in range(4):
                                nc.tensor.matmul(
                                    av3[:, ct, :],
                                    phat[p][mt][:, 128 * ct : 128 * (ct + 1)],
                                    vaug_sb[p][mt][:],
                                    start=(mt == 0),
                                    stop=(mt == 3),
                                )
                        rc = stage.tile([128, 4], F32, name="rc", tag="rc", bufs=4)
                        nc.vector.reciprocal(rc[:], av3[:, :, 64])
                        for ct in range(4):
                            o3 = out2dT[p][ct].rearrange(
                                "p (d s) -> p d s", d=64, s=8
                            )
                            nc.vector.tensor_scalar_mul(
                                o3[:, :, ch], av3[:, ct, 0:64], rc[:, ct : ct + 1]
                            )

                # ---------------- projection + output ----------------
                for p in range(2):
                    for rt in range(4):
                        pr_ps = ps512.tile([128, 512], F32, name="pr_ps", tag="mm512")
                        for ct in range(4):
                            nc.tensor.matmul(
                                pr_ps[:],
                                out2dT[p][ct][:, 128 * rt : 128 * (rt + 1)],
                                wproj_sb[ct][:],
                                start=(ct == 0),
                                stop=False,
                            )
                        nc.tensor.matmul(
                            pr_ps[:], crow(ROW_ONES, 128), crow(ROW_BPROJ),
                            start=False, stop=True,
                        )
                        of = stage.tile([128, 512], F32, name="of", tag="of", bufs=3)
                        nc.scalar.activation(of[:], pr_ps[:], AF.Copy)
                        r0 = 512 * p + 128 * rt
                        nc.sync.dma_start(out[r0 : r0 + 128, :], of[:])

    nc.compile()
    return nc


_NC_CACHE = None


def _get_module():
    global _NC_CACHE
    if _NC_CACHE is None:
        _NC_CACHE = _build_module()
    return _NC_CACHE


def _prep_core_inputs(inputs):
    """Host-side sharding: layout/permute/cast weights, build 8 in_maps."""
    x = np.asarray(inputs["x"], np.float32)
    Wq = np.asarray(inputs["Wq"], np.float32)
    bq = np.asarray(inputs["bq"], np.float32)
    Wconv = np.asarray(inputs["Wconv"], np.float32)
    bconv = np.asarray(inputs["bconv"], np.float32)
    gamma = np.asarray(inputs["gamma"], np.float32)
    beta = np.asarray(inputs["beta"], np.float32)
    Wkv = np.asarray(inputs["Wkv"], np.float32)
    bkv = np.asarray(inputs["bkv"], np.float32)
    Wproj = np.asarray(inputs["Wproj"], np.float32)
    bproj = np.asarray(inputs["bproj"], np.float32)

    # Xp: [ic, (m, tap)]; n = 128i + 64di + 2j + dj, m = 32i+j, tap = 2di+dj
    xp_g = []
    for b in range(B):
        xt = x[b].T.reshape(C, 32, 2, 32, 2)  # [ic, i, di, j, dj]
        xt = np.ascontiguousarray(
            xt.transpose(0, 1, 3, 2, 4).reshape(C, 8, 512)  # [ic, blk, rest]
        )
        xp_g.append(xt)

    wconvt = np.ascontiguousarray(
        Wconv.transpose(1, 2, 3, 0).reshape(C, 4, C)
    ).astype(NP_BF16)

    wkvp = gamma[:, None] * Wkv
    s_full = beta @ Wkv + bkv

    # Wproj row permutation: u' = 128 i' + 4 j + 2 di + dj -> n' = 128 i' + 64 di + 2 j + dj
    up = np.arange(C)
    i_, j_ = up // 128, (up % 128) // 4
    di, dj = (up % 4) // 2, up % 2
    nprime = 128 * i_ + 64 * di + 2 * j_ + dj
    wproj_perm = np.ascontiguousarray(Wproj[nprime, :]).astype(NP_BF16)

    eye128 = np.eye(128, dtype=np.float32).astype(NP_BF16)
    eyef = np.eye(8, dtype=np.float32)

    in_maps = []
    for core in range(N_CORES):
        b, g = divmod(core, 4)
        kcols = slice(128 * g, 128 * (g + 1))
        vcols = slice(512 + 128 * g, 512 + 128 * (g + 1))
        xp_loc = np.ascontiguousarray(xp_g[b].reshape(C, N)).astype(NP_BF16)
        small = np.zeros((128, 2), np.float32)
        small[:, 0] = bq[kcols]
        rows16 = np.zeros((8, C), np.float32)
        rows16[ROW_BCONV] = bconv
        rows16[ROW_BPROJ] = bproj
        rows16[ROW_TK, 0:128] = wkvp[:, kcols].sum(0)
        rows16[ROW_TV, 0:128] = wkvp[:, vcols].sum(0)
        rows16[ROW_SK, 0:128] = s_full[kcols]
        rows16[ROW_SV, 0:128] = s_full[vcols]
        rows16[ROW_ONES] = 1.0
        wkv2 = np.concatenate([wkvp[:, kcols], wkvp[:, vcols]], axis=1)
        in_maps.append(
            {
                "xp": xp_loc,
                "wq": np.ascontiguousarray(Wq[:, kcols]).astype(NP_BF16),
                "wconvt": wconvt,
                "wkv2": np.ascontiguousarray(wkv2).astype(NP_BF16),
                "wproj": wproj_perm,
                "small": small,
                "rows16": rows16.reshape(1, 8 * C).astype(NP_BF16),
                "eye128": eye128,
                "eyef": eyef,
            }
        )
    return in_maps


def run_spmd(inputs, **kwargs):
    """Run the SPMD kernel; returns (full_output, BassKernelResults)."""
    nc = _get_module()
    in_maps = _prep_core_inputs(inputs)
    res = run_bass_kernel_spmd(nc, in_maps, core_ids=list(range(N_CORES)), **kwargs)
    full = np.empty((B, N, C), np.float32)
    for core in range(N_CORES):
        b, g = divmod(core, 4)
        full[b, 1024 * g : 1024 * (g + 1), :] = res.results[core]["out"]
    return full, res


def kernel(**inputs) -> np.ndarray:
    full, _ = run_spmd(inputs)
    return full


# revision 15
# speedup vs baseline: 74.6526x; 1.1773x over previous
"""Trainium2 Bass kernel for PVT-style spatial-reduction multi-head attention.

Problem (hardcoded shapes, fp32 inputs):
  x [2, 4096, 512]; Wq [512,512]; Wconv [512,512,2,2] (OIHW, stride 2);
  LayerNorm over the conv's flattened spatial dim (M=1024); Wkv [1024,1024];
  attention with q [B,8,4096,64], k/v [B,8,512,64]; "faithful" reshape
  (out.transpose(0,1,3,2).reshape(B,-1,512)) before Wproj [512,512].

Sharding: 8 cores = (batch b in {0,1}) x (head-pair g in {0..3}).
Core (b,g) computes heads {2g, 2g+1} of batch b and writes output rows
[b, 1024g : 1024g+1024, :].

v3 design vs the v2 baseline (113 us):
 - x is sent host-side in a tap-expanded transposed layout Xp[ic, (m, tap)]
   (n = 128i + 64di + 2j + dj; m = 32i + j; tap = 2di + dj), so there is no
   on-chip x transpose. Q and the conv consume Xp directly; the resulting
   within-chunk column permutation of q (and of the attention output) is
   absorbed into a host-side row permutation of Wproj.
 - The stride-2 2x2 VALID conv is non-overlapping and is computed
   TRANSPOSED (xcT [m, o] = Xp^T @ Wconv'), which feeds the KV matmul with
   no transpose stage. (A 4-way m-sharded variant with a DRAM AllGather was
   tried and reverted: one collective_compute costs ~150-250 us through
   this NRT path, dwarfing the 20 us of saved conv work.)
 - LayerNorm is folded algebraically: gamma into Wkv rows (host), beta+bkv
   into a bias row s (host), so kv_noscale = xcT @ Wkv' - mu_c t + s*sqrtve_c
   with the mu/s terms as K=1 rank-1 PE matmuls into the same PSUM group.
   The per-position scale rs_c = rsqrt(var_c+eps) folds into the softmax
   exp scale (k side, per-partition AP scale) and the vaug scale (v side).
   Stats (sum x, sum x^2) come from ones-vector PE matmuls; rsqrt is
   computed as exp(-0.5*ln(v)) so the ACT engine stays on the single
   activation table holding exp/ln/copy/square/identity (no 1283ns act
   table reloads anywhere in the kernel).
 - AV is computed n-partitioned (av2[n, d] = sum_c phat[c,n]^T [v^T*rs|1]),
   so there is no AV transpose stage, and softmax denominators ride along
   as a 65th rhs column, landing pre-transposed for the normalize.
 - Engine balance: ACT does exp + PSUM->SBUF copies (one act table), DVE
   does Q epilogue, squares, AV normalize; reciprocal+normalize are
   interleaved into the chunk loop so only the projection remains as tail.
"""

import sys

sys.path.insert(0, "/opt/trn_rl_repo")

import math

import numpy as np
import ml_dtypes

import concourse.bass as bass
import concourse.bacc as bacc
import concourse.mybir as mybir
import concourse.tile as tile
from concourse.bass_utils import run_bass_kernel_spmd

F32 = mybir.dt.float32
BF16 = mybir.dt.bfloat16
NP_BF16 = ml_dtypes.bfloat16

B, N, C = 2, 4096, 512
NH, HD, SR = 8, 64, 2
M = 1024
EPS = 1e-5
N_CORES = 8

# rows16 row map ([8, 512] bf16 host constants)
ROW_BCONV = 0
ROW_BPROJ = 1
ROW_TK = 2
ROW_TV = 3
ROW_SK = 4
ROW_SV = 5
ROW_ONES = 6


def _build_module(reps=1, bench_internal=False):
    nc = bacc.Bacc("TRN2", target_bir_lowering=False, debug=False)

    # ---- per-core DRAM tensors (data differs per core, shapes identical) ----
    KIND = "Internal" if bench_internal else "ExternalInput"
    xp = nc.dram_tensor("xp", [C, N], BF16, kind=KIND).ap()
    wq = nc.dram_tensor("wq", [C, 128], BF16, kind=KIND).ap()
    wconvt = nc.dram_tensor("wconvt", [C, 4, C], BF16, kind=KIND).ap()
    wkv2 = nc.dram_tensor("wkv2", [M, 256], BF16, kind=KIND).ap()
    wproj = nc.dram_tensor("wproj", [C, C], BF16, kind=KIND).ap()
    small = nc.dram_tensor("small", [128, 2], F32, kind="ExternalInput").ap()
    rows16 = nc.dram_tensor("rows16", [1, 8 * C], BF16, kind=KIND).ap()
    eye128 = nc.dram_tensor("eye128", [128, 128], BF16, kind=KIND).ap()
    eyef = nc.dram_tensor("eyef", [8, 8], F32, kind=KIND).ap()
    OKIND = "Internal" if bench_internal else "ExternalOutput"
    out = nc.dram_tensor("out", [2 * C, C], F32, kind=OKIND).ap()
    dummy = (
        nc.dram_tensor("bench_out", [1, 2], F32, kind="ExternalOutput").ap()
        if bench_internal
        else None
    )

    AX = mybir.AxisListType.X
    OP = mybir.AluOpType
    AF = mybir.ActivationFunctionType
    LN8 = math.log(0.125)

    with tile.TileContext(nc) as tc:
        import contextlib

        with contextlib.ExitStack() as ctx:
            persist = ctx.enter_context(tc.tile_pool(name="persist", bufs=1))
            stage = ctx.enter_context(tc.tile_pool(name="stage", bufs=3))
            ps512 = ctx.enter_context(tc.tile_pool(name="ps512", bufs=3, space="PSUM"))
            psa2 = ctx.enter_context(tc.tile_pool(name="psa2", bufs=3, space="PSUM"))
            pstp = ctx.enter_context(tc.tile_pool(name="pstp", bufs=2, space="PSUM"))

            for _rep in range(reps):
                # ---------------- weight / const loads ----------------
                wq_sb = []
                for k in range(4):
                    t = persist.tile([128, 128], BF16, name=f"wq_sb{k}", tag=f"wq{k}")
                    nc.sync.dma_start(t[:], wq[128 * k : 128 * (k + 1), :])
                    wq_sb.append(t)

                wconv_sb = []  # [ic_t] -> [128 ic, (tap 4, o 512)]
                for kt in range(4):
                    t = persist.tile([128, 2048], BF16, name=f"wconv{kt}", tag=f"wc{kt}")
                    nc.sync.dma_start(t[:], wconvt[128 * kt : 128 * (kt + 1), :, :])
                    wconv_sb.append(t)

                wkv_sb = []  # [mt] -> [128 m, 256] (cols 0:128 k, 128:256 v)
                for k in range(8):
                    t = persist.tile([128, 256], BF16, name=f"wkv_sb{k}", tag=f"wkv{k}")
                    nc.sync.dma_start(t[:], wkv2[128 * k : 128 * (k + 1), :])
                    wkv_sb.append(t)

                wproj_sb = []
                for ct in range(4):
                    t = persist.tile([128, 512], BF16, name=f"wproj{ct}", tag=f"wp{ct}")
                    nc.sync.dma_start(t[:], wproj[128 * ct : 128 * (ct + 1), :])
                    wproj_sb.append(t)

                small_sb = persist.tile([128, 2], F32, name="small_sb", tag="small")
                nc.sync.dma_start(small_sb[:], small[:, :])
                if dummy is not None and _rep == 0:
                    nc.sync.dma_start(dummy[:, :], small[0:1, 0:2])
                rows_sb = persist.tile([1, 4096], BF16, name="rows_sb", tag="rows16")
                nc.sync.dma_start(rows_sb[:], rows16[:, :])

                def crow(r, n=512):
                    return rows_sb[0:1, 512 * r : 512 * r + n]
                eye_sb = persist.tile([128, 128], BF16, name="eye_sb", tag="eye128")
                nc.sync.dma_start(eye_sb[:], eye128[:, :])
                eyef_sb = persist.tile([8, 8], F32, name="eyef_sb", tag="eyef")
                nc.sync.dma_start(eyef_sb[:], eyef[:, :])
                onec_sb = persist.tile([128, 1], BF16, name="onec_sb", tag="onec")
                nc.vector.memset(onec_sb[:], 1.0)
                ln8_sb = persist.tile([128, 1], F32, name="ln8_sb", tag="ln8")
                nc.vector.memset(ln8_sb[:], LN8)

                bq_col = small_sb[:, 0:1]

                # ---------------- x load (conv slices first) ----------------
                xp_sb = []  # [ic_t] -> [128 ic, 4096 (m,tap)]
                for kt in range(4):
                    t = persist.tile([128, N], BF16, name=f"xp_sb{kt}", tag=f"xp{kt}")
                    xp_sb.append(t)
                for kt in range(4):
                    nc.sync.dma_start(
                        xp_sb[kt][:, 0:1024], xp[128 * kt : 128 * (kt + 1), 0:1024]
                    )
                for kt in range(4):
                    nc.sync.dma_start(
                        xp_sb[kt][:, 1024:4096], xp[128 * kt : 128 * (kt + 1), 1024:4096]
                    )
                xp4 = [t.rearrange("p (m tap) -> p m tap", m=M, tap=4) for t in xp_sb]

                # ---------------- conv (all 8 m-blocks), stats ----------------
                xcl_sb = []  # conv out [128 m, 512 o] bf16 per m-block
                sq_sb = []
                for l in range(8):
                    c_ps = ps512.tile([128, 512], F32, name="c_ps", tag="mm512")
                    first = True
                    for kt in range(4):
                        for tap in range(4):
                            nc.tensor.matmul(
                                c_ps[:],
                                xp4[kt][:, 128 * l : 128 * (l + 1), tap],
                                wconv_sb[kt][:, 512 * tap : 512 * (tap + 1)],
                                start=first,
                                stop=False,
                            )
                            first = False
                    # += ones (x) bconv  (rank-1 bias over m partitions)
                    nc.tensor.matmul(
                        c_ps[:], crow(ROW_ONES, 128), crow(ROW_BCONV),
                        start=False, stop=True,
                    )
                    xcl = persist.tile([128, 512], BF16, name=f"xcl{l}", tag=f"xcl{l}")
                    nc.scalar.activation(xcl[:], c_ps[:], AF.Copy)
                    sq = persist.tile([128, 512], BF16, name=f"sq{l}", tag=f"sql{l}")
                    nc.vector.tensor_mul(sq[:], xcl[:], xcl[:])
                    xcl_sb.append(xcl)
                    sq_sb.append(sq)
                sx_ps = ps512.tile([1, 512], F32, name="sx_ps", tag="mm512")
                sq_ps = ps512.tile([1, 512], F32, name="sq_ps", tag="mm512")
                for l in range(8):
                    nc.tensor.matmul(
                        sx_ps[:], onec_sb[:], xcl_sb[l][:],
                        start=(l == 0), stop=(l == 7),
                    )
                    nc.tensor.matmul(
                        sq_ps[:], onec_sb[:], sq_sb[l][:],
                        start=(l == 0), stop=(l == 7),
                    )
                srow_sb = persist.tile([1, 1024], F32, name="srow_sb", tag="srow")
                nc.vector.tensor_copy(srow_sb[0:1, 0:512], sx_ps[:])
                nc.vector.tensor_copy(srow_sb[0:1, 512:1024], sq_ps[:])
                sx_row = srow_sb[0:1, 0:512]
                sq_row = srow_sb[0:1, 512:1024]

                # ---------------- Q projection ----------------
                qt_sb = persist.tile([128, N], BF16, name="qt_sb", tag="qt")
                for ch in range(8):
                    q_ps = ps512.tile([128, 512], F32, name="q_ps", tag="mm512")
                    for k in range(4):
                        nc.tensor.matmul(
                            q_ps[:],
                            wq_sb[k][:],
                            xp4[k][:, 128 * ch : 128 * (ch + 1), :],
                            start=(k == 0),
                            stop=(k == 3),
                        )
                    nc.vector.tensor_scalar_add(
                        qt_sb[:, 512 * ch : 512 * (ch + 1)], q_ps[:], bq_col
                    )

                # ---------------- stats math ----------------
                murow = stage.tile([1, 512], F32, name="murow", tag="murow", bufs=1)
                nc.vector.tensor_scalar_mul(murow[:], sx_row, 1.0 / M)
                negmu16 = persist.tile([1, 512], BF16, name="negmu16", tag="negmu")
                nc.vector.tensor_scalar_mul(negmu16[:], sx_row, -1.0 / M)
                verow = persist.tile([1, 512], F32, name="verow", tag="verow")
                nc.vector.tensor_scalar(
                    out=verow[:], in0=sq_row,
                    scalar1=1.0 / M, scalar2=EPS, op0=OP.mult, op1=OP.add,
                )
                mu2 = stage.tile([1, 512], F32, name="mu2", tag="mu2", bufs=1)
                nc.vector.tensor_mul(mu2[:], murow[:], murow[:])
                nc.vector.tensor_sub(verow[:], verow[:], mu2[:])
                # sqrtve row (bf16) = exp(0.5 ln ve)
                lrow = stage.tile([1, 512], F32, name="lrow", tag="lrow", bufs=1)
                nc.scalar.activation(lrow[:], verow[:], AF.Ln)
                sqve16 = persist.tile([1, 512], BF16, name="sqve16", tag="sqve")
                nc.scalar.activation(sqve16[:], lrow[:], AF.Exp, scale=0.5)
                # columns: ve -> [128, 4] via PE transpose, then exp/ln scales
                vecol_ps = ps512.tile([128, 4], F32, name="vecol_ps", tag="mm512")
                for j in range(4):
                    nc.tensor.transpose(
                        vecol_ps[:, j : j + 1],
                        verow[:, 128 * j : 128 * (j + 1)],
                        eyef_sb[0:1, 0:1],
                    )
                lcol = stage.tile([128, 4], F32, name="lcol", tag="lcol", bufs=1)
                nc.scalar.activation(lcol[:], vecol_ps[:], AF.Ln)
                esc_col = persist.tile([128, 4], F32, name="esc_col", tag="esc")
                nc.scalar.activation(esc_col[:], lcol[:], AF.Exp, scale=-0.5, bias=ln8_sb[:])
                vsc_col = persist.tile([128, 4], F32, name="vsc_col", tag="vsc")
                nc.scalar.activation(vsc_col[:], lcol[:], AF.Exp, scale=-0.5)

                # ---------------- KV ----------------
                kT_sb = persist.tile([128, 512], BF16, name="kT_sb", tag="kT")
                vT_sb = persist.tile([128, 512], BF16, name="vT_sb", tag="vT")
                for which, lo, t_row, s_row, dst in (
                    ("k", 0, ROW_TK, ROW_SK, kT_sb),
                    ("v", 128, ROW_TV, ROW_SV, vT_sb),
                ):
                    kv_ps = ps512.tile([128, 512], F32, name="kv_ps", tag="mm512")
                    for k in range(8):
                        nc.tensor.matmul(
                            kv_ps[:], wkv_sb[k][:, lo : lo + 128], xcl_sb[k][:],
                            start=(k == 0), stop=False,
                        )
                    nc.tensor.matmul(
                        kv_ps[:], crow(t_row, 128), negmu16[:],
                        start=False, stop=False,
                    )
                    nc.tensor.matmul(
                        kv_ps[:], crow(s_row, 128), sqve16[:],
                        start=False, stop=True,
                    )
                    nc.scalar.activation(dst[:], kv_ps[:], AF.Copy)

                # vaug[p][mt]: [128 c, 64 d] bf16, v^T with rs folded
                vaug_sb = []
                for p in range(2):
                    row = []
                    for mt in range(4):
                        t = persist.tile(
                            [128, 65], BF16, name=f"vaug{p}_{mt}", tag=f"va{p}{mt}"
                        )
                        tp = pstp.tile([128, 64], BF16, name="tpv", tag="ptp")
                        nc.tensor.transpose(
                            tp[:],
                            vT_sb[64 * p : 64 * (p + 1), 128 * mt : 128 * (mt + 1)],
                            eye_sb[64 * p : 64 * (p + 1), 64 * p : 64 * (p + 1)],
                        )
                        nc.vector.tensor_scalar_mul(
                            t[:, 0:64], tp[:], vsc_col[:, mt : mt + 1]
                        )
                        nc.vector.memset(t[:, 64:65], 1.0)
                        row.append(t)
                    vaug_sb.append(row)

                # ---------------- attention ----------------
                # AV is computed n-partitioned: av2[n, (d | den)] = sum_c
                # phat[c, n]^T @ [v^T*rs | 1], so no AV transpose stage and
                # denominators ride along as column 64 of each ct block.
                out2dT = []
                for p in range(2):
                    o_row = []
                    for ct in range(4):
                        t = persist.tile(
                            [128, 512], BF16, name=f"o2dT{p}_{ct}", tag=f"o2{p}{ct}"
                        )
                        o_row.append(t)
                    out2dT.append(o_row)

                def emit_s(ch, phat_all):
                    phat = {0: [], 1: []}
                    for mt in range(4):
                        for p in range(2):
                            s_ps = ps512.tile([128, 512], F32, name="s_ps", tag="mm512")
                            nc.tensor.matmul(
                                s_ps[:],
                                kT_sb[64 * p : 64 * (p + 1), 128 * mt : 128 * (mt + 1)],
                                qt_sb[64 * p : 64 * (p + 1), 512 * ch : 512 * (ch + 1)],
                                start=True,
                                stop=True,
                            )
                            ph = stage.tile(
                                [128, 512], BF16, name="phat", tag="phat", bufs=16
                            )
                            nc.scalar.activation(
                                ph[:], s_ps[:], AF.Exp, scale=esc_col[:, mt : mt + 1]
                            )
                            phat[p].append(ph)
                    phat_all[ch] = phat

                def emit_av(ch, phat_all):
                    phat = phat_all.pop(ch)
                    for p in range(2):
                        av2 = psa2.tile([128, 260], F32, name="av2", tag="psa2")
                        av3 = av2.rearrange("p (ct u) -> p ct u", ct=4, u=65)
                        for ct in range(4):
                            for mt in range(4):
                                nc.tensor.matmul(
                                    av3[:, ct, :],
                                    phat[p][mt][:, 128 * ct : 128 * (ct + 1)],
                                    vaug_sb[p][mt][:],
                                    start=(mt == 0),
                                    stop=(mt == 3),
                                )
                        rc = stage.tile([128, 4], F32, name="rc", tag="rc", bufs=4)
                        nc.vector.reciprocal(rc[:], av3[:, :, 64])
                        for ct in range(4):
                            o3 = out2dT[p][ct].rearrange(
                                "p (d s) -> p d s", d=64, s=8
                            )
                            nc.vector.tensor_scalar_mul(
                                o3[:, :, ch], av3[:, ct, 0:64], rc[:, ct : ct + 1]
                            )

                # software pipeline: S/exp of chunk ch+1 is emitted before
                # AV of chunk ch so the in-order PE never sits on an AV
                # matmul waiting for exp to drain.
                phat_all = {}
                emit_s(0, phat_all)
                for ch in range(8):
                    if ch + 1 < 8:
                        emit_s(ch + 1, phat_all)
                    emit_av(ch, phat_all)

                # ---------------- projection + output ----------------
                for p in range(2):
                    for rt in range(4):
                        pr_ps = ps512.tile([128, 512], F32, name="pr_ps", tag="mm512")
                        for ct in range(4):
                            nc.tensor.matmul(
                                pr_ps[:],
                                out2dT[p][ct][:, 128 * rt : 128 * (rt + 1)],
                                wproj_sb[ct][:],
                                start=(ct == 0),
                                stop=False,
                            )
                        nc.tensor.matmul(
                            pr_ps[:], crow(ROW_ONES, 128), crow(ROW_BPROJ),
                            start=False, stop=True,
                        )
                        of = stage.tile([128, 512], F32, name="of", tag="of", bufs=3)
                        nc.scalar.activation(of[:], pr_ps[:], AF.Copy)
                        r0 = 512 * p + 128 * rt
                        nc.sync.dma_start(out[r0 : r0 + 128, :], of[:])

    nc.compile()
    return nc


_NC_CACHE = None


def _get_module():
    global _NC_CACHE
    if _NC_CACHE is None:
        _NC_CACHE = _build_module()
    return _NC_CACHE


def _prep_core_inputs(inputs):
    """Host-side sharding: layout/permute/cast weights, build 8 in_maps."""
    x = np.asarray(inputs["x"], np.float32)
    Wq = np.asarray(inputs["Wq"], np.float32)
    bq = np.asarray(inputs["bq"], np.float32)
    Wconv = np.asarray(inputs["Wconv"], np.float32)
    bconv = np.asarray(inputs["bconv"], np.float32)
    gamma = np.asarray(inputs["gamma"], np.float32)
    beta = np.asarray(inputs["beta"], np.float32)
    Wkv = np.asarray(inputs["Wkv"], np.float32)
    bkv = np.asarray(inputs["bkv"], np.float32)
    Wproj = np.asarray(inputs["Wproj"], np.float32)
    bproj = np.asarray(inputs["bproj"], np.float32)

    # Xp: [ic, (m, tap)]; n = 128i + 64di + 2j + dj, m = 32i+j, tap = 2di+dj
    xp_g = []
    for b in range(B):
        xt = x[b].T.reshape(C, 32, 2, 32, 2)  # [ic, i, di, j, dj]
        xt = np.ascontiguousarray(
            xt.transpose(0, 1, 3, 2, 4).reshape(C, 8, 512)  # [ic, blk, rest]
        )
        xp_g.append(xt)

    wconvt = np.ascontiguousarray(
        Wconv.transpose(1, 2, 3, 0).reshape(C, 4, C)
    ).astype(NP_BF16)

    wkvp = gamma[:, None] * Wkv
    s_full = beta @ Wkv + bkv

    # Wproj row permutation: u' = 128 i' + 4 j + 2 di + dj -> n' = 128 i' + 64 di + 2 j + dj
    up = np.arange(C)
    i_, j_ = up // 128, (up % 128) // 4
    di, dj = (up % 4) // 2, up % 2
    nprime = 128 * i_ + 64 * di + 2 * j_ + dj
    wproj_perm = np.ascontiguousarray(Wproj[nprime, :]).astype(NP_BF16)

    eye128 = np.eye(128, dtype=np.float32).astype(NP_BF16)
    eyef = np.eye(8, dtype=np.float32)

    in_maps = []
    for core in range(N_CORES):
        b, g = divmod(core, 4)
        kcols = slice(128 * g, 128 * (g + 1))
        vcols = slice(512 + 128 * g, 512 + 128 * (g + 1))
        xp_loc = np.ascontiguousarray(xp_g[b].reshape(C, N)).astype(NP_BF16)
        small = np.zeros((128, 2), np.float32)
        small[:, 0] = bq[kcols]
        rows16 = np.zeros((8, C), np.float32)
        rows16[ROW_BCONV] = bconv
        rows16[ROW_BPROJ] = bproj
        rows16[ROW_TK, 0:128] = wkvp[:, kcols].sum(0)
        rows16[ROW_TV, 0:128] = wkvp[:, vcols].sum(0)
        rows16[ROW_SK, 0:128] = s_full[kcols]
        rows16[ROW_SV, 0:128] = s_full[vcols]
        rows16[ROW_ONES] = 1.0
        wkv2 = np.concatenate([wkvp[:, kcols], wkvp[:, vcols]], axis=1)
        in_maps.append(
            {
                "xp": xp_loc,
                "wq": np.ascontiguousarray(Wq[:, kcols]).astype(NP_BF16),
                "wconvt": wconvt,
                "wkv2": np.ascontiguousarray(wkv2).astype(NP_BF16),
                "wproj": wproj_perm,
                "small": small,
                "rows16": rows16.reshape(1, 8 * C).astype(NP_BF16),
                "eye128": eye128,
                "eyef": eyef,
            }
        )
    return in_maps


def run_spmd(inputs, **kwargs):
    """Run the SPMD kernel; returns (full_output, BassKernelResults)."""
    nc = _get_module()
    in_maps = _prep_core_inputs(inputs)
    res = run_bass_kernel_spmd(nc, in_maps, core_ids=list(range(N_CORES)), **kwargs)
    full = np.empty((B, N, C), np.float32)
    for core in range(N_CORES):
        b, g = divmod(core, 4)
        full[b, 1024 * g : 1024 * (g + 1), :] = res.results[core]["out"]
    return full, res


def kernel(**inputs) -> np.ndarray:
    full, _ = run_spmd(inputs)
    return full


# revision 18
# speedup vs baseline: 121.2466x; 1.6241x over previous
"""Trainium2 Bass kernel for PVT-style spatial-reduction multi-head attention.

Problem (hardcoded shapes, fp32 inputs):
  x [2, 4096, 512]; Wq [512,512]; Wconv [512,512,2,2] (OIHW, stride 2);
  LayerNorm over the conv's flattened spatial dim (M=1024); Wkv [1024,1024];
  attention with q [B,8,4096,64], k/v [B,8,512,64]; "faithful" reshape
  (out.transpose(0,1,3,2).reshape(B,-1,512)) before Wproj [512,512].

Sharding: 8 cores = (batch b in {0,1}) x (head-pair g in {0..3}).
Core (b,g) computes heads {2g, 2g+1} of batch b and writes output rows
[b, 1024g : 1024g+1024, :].

v3 design vs the v2 baseline (113 us):
 - x is sent host-side in a tap-expanded transposed layout Xp[ic, (m, tap)]
   (n = 128i + 64di + 2j + dj; m = 32i + j; tap = 2di + dj), so there is no
   on-chip x transpose. Q and the conv consume Xp directly; the resulting
   within-chunk column permutation of q (and of the attention output) is
   absorbed into a host-side row permutation of Wproj.
 - The stride-2 2x2 VALID conv is non-overlapping and is computed
   TRANSPOSED (xcT [m, o] = Xp^T @ Wconv'), which feeds the KV matmul with
   no transpose stage. (A 4-way m-sharded variant with a DRAM AllGather was
   tried and reverted: one collective_compute costs ~150-250 us through
   this NRT path, dwarfing the 20 us of saved conv work.)
 - LayerNorm is folded algebraically: gamma into Wkv rows (host), beta+bkv
   into a bias row s (host), so kv_noscale = xcT @ Wkv' - mu_c t + s*sqrtve_c
   with the mu/s terms as K=1 rank-1 PE matmuls into the same PSUM group.
   The per-position scale rs_c = rsqrt(var_c+eps) folds into the softmax
   exp scale (k side, per-partition AP scale) and the vaug scale (v side).
   Stats (sum x, sum x^2) come from ones-vector PE matmuls; rsqrt is
   computed as exp(-0.5*ln(v)) so the ACT engine stays on the single
   activation table holding exp/ln/copy/square/identity (no 1283ns act
   table reloads anywhere in the kernel).
 - AV is computed n-partitioned (av2[n, d] = sum_c phat[c,n]^T [v^T*rs|1]),
   so there is no AV transpose stage, and softmax denominators ride along
   as a 65th rhs column, landing pre-transposed for the normalize.
 - Engine balance: ACT does exp + PSUM->SBUF copies (one act table), DVE
   does Q epilogue, squares, AV normalize; reciprocal+normalize are
   interleaved into the chunk loop so only the projection remains as tail.
"""

import sys

sys.path.insert(0, "/opt/trn_rl_repo")

import math

import numpy as np
import ml_dtypes

import concourse.bass as bass
import concourse.bacc as bacc
import concourse.mybir as mybir
import concourse.tile as tile
from concourse.bass_utils import run_bass_kernel_spmd

F32 = mybir.dt.float32
BF16 = mybir.dt.bfloat16
NP_BF16 = ml_dtypes.bfloat16

B, N, C = 2, 4096, 512
NH, HD, SR = 8, 64, 2
M = 1024
EPS = 1e-5
N_CORES = 8

# rows16 row map ([8, 512] bf16 host constants)
ROW_BCONV = 0
ROW_BPROJ = 1
ROW_TK = 2
ROW_TV = 3
ROW_SK = 4
ROW_SV = 5
ROW_ONES = 6


def _build_module(reps=1, bench_internal=False):
    nc = bacc.Bacc("TRN2", target_bir_lowering=False, debug=False)

    # ---- per-core DRAM tensors (data differs per core, shapes identical) ----
    KIND = "Internal" if bench_internal else "ExternalInput"
    xp = nc.dram_tensor("xp", [C, N], BF16, kind=KIND).ap()
    wq = nc.dram_tensor("wq", [C, 128], BF16, kind=KIND).ap()
    wconvt = nc.dram_tensor("wconvt", [C, 4, C], BF16, kind=KIND).ap()
    wkv2 = nc.dram_tensor("wkv2", [M, 256], BF16, kind=KIND).ap()
    wproj = nc.dram_tensor("wproj", [C, C], BF16, kind=KIND).ap()
    small = nc.dram_tensor("small", [128, 2], F32, kind="ExternalInput").ap()
    rows16 = nc.dram_tensor("rows16", [1, 8 * C], BF16, kind=KIND).ap()
    eye128 = nc.dram_tensor("eye128", [128, 128], BF16, kind=KIND).ap()
    eyef = nc.dram_tensor("eyef", [8, 8], F32, kind=KIND).ap()
    OKIND = "Internal" if bench_internal else "ExternalOutput"
    out = nc.dram_tensor("out", [2 * C, C], F32, kind=OKIND).ap()
    dummy = (
        nc.dram_tensor("bench_out", [1, 2], F32, kind="ExternalOutput").ap()
        if bench_internal
        else None
    )

    AX = mybir.AxisListType.X
    OP = mybir.AluOpType
    AF = mybir.ActivationFunctionType
    LN8 = math.log(0.125)

    with tile.TileContext(nc) as tc:
        import contextlib

        with contextlib.ExitStack() as ctx:
            persist = ctx.enter_context(tc.tile_pool(name="persist", bufs=1))
            stage = ctx.enter_context(tc.tile_pool(name="stage", bufs=3))
            ps512 = ctx.enter_context(tc.tile_pool(name="ps512", bufs=3, space="PSUM"))
            psa2 = ctx.enter_context(tc.tile_pool(name="psa2", bufs=3, space="PSUM"))
            pstp = ctx.enter_context(tc.tile_pool(name="pstp", bufs=2, space="PSUM"))

            for _rep in range(reps):
                # ---------------- weight / const loads ----------------
                wq_sb = []
                for k in range(4):
                    t = persist.tile([128, 128], BF16, name=f"wq_sb{k}", tag=f"wq{k}")
                    nc.sync.dma_start(t[:], wq[128 * k : 128 * (k + 1), :])
                    wq_sb.append(t)

                wconv_sb = []  # [ic_t] -> [128 ic, (tap 4, o 512)]
                for kt in range(4):
                    t = persist.tile([128, 2048], BF16, name=f"wconv{kt}", tag=f"wc{kt}")
                    nc.sync.dma_start(t[:], wconvt[128 * kt : 128 * (kt + 1), :, :])
                    wconv_sb.append(t)

                wkv_sb = []  # [mt] -> [128 m, 256] (cols 0:128 k, 128:256 v)
                for k in range(8):
                    t = persist.tile([128, 256], BF16, name=f"wkv_sb{k}", tag=f"wkv{k}")
                    nc.sync.dma_start(t[:], wkv2[128 * k : 128 * (k + 1), :])
                    wkv_sb.append(t)

                wproj_sb = []
                for ct in range(4):
                    t = persist.tile([128, 512], BF16, name=f"wproj{ct}", tag=f"wp{ct}")
                    nc.sync.dma_start(t[:], wproj[128 * ct : 128 * (ct + 1), :])
                    wproj_sb.append(t)

                small_sb = persist.tile([128, 2], F32, name="small_sb", tag="small")
                nc.sync.dma_start(small_sb[:], small[:, :])
                if dummy is not None and _rep == 0:
                    nc.sync.dma_start(dummy[:, :], small[0:1, 0:2])
                rows_sb = persist.tile([1, 4096], BF16, name="rows_sb", tag="rows16")
                nc.sync.dma_start(rows_sb[:], rows16[:, :])

                def crow(r, n=512):
                    return rows_sb[0:1, 512 * r : 512 * r + n]
                eye_sb = persist.tile([128, 128], BF16, name="eye_sb", tag="eye128")
                nc.sync.dma_start(eye_sb[:], eye128[:, :])
                eyef_sb = persist.tile([8, 8], F32, name="eyef_sb", tag="eyef")
                nc.sync.dma_start(eyef_sb[:], eyef[:, :])
                onec_sb = persist.tile([128, 1], BF16, name="onec_sb", tag="onec")
                nc.vector.memset(onec_sb[:], 1.0)
                ln8_sb = persist.tile([128, 1], F32, name="ln8_sb", tag="ln8")
                nc.vector.memset(ln8_sb[:], LN8)

                bq_col = small_sb[:, 0:1]

                # ---------------- x load (conv slices first) ----------------
                xp_sb = []  # [ic_t] -> [128 ic, 4096 (m,tap)]
                for kt in range(4):
                    t = persist.tile([128, N], BF16, name=f"xp_sb{kt}", tag=f"xp{kt}")
                    xp_sb.append(t)
                for kt in range(4):
                    nc.sync.dma_start(
                        xp_sb[kt][:, 0:1024], xp[128 * kt : 128 * (kt + 1), 0:1024]
                    )
                for kt in range(4):
                    nc.sync.dma_start(
                        xp_sb[kt][:, 1024:4096], xp[128 * kt : 128 * (kt + 1), 1024:4096]
                    )
                xp4 = [t.rearrange("p (m tap) -> p m tap", m=M, tap=4) for t in xp_sb]

                # ---------------- conv (all 8 m-blocks), stats ----------------
                xcl_sb = []  # conv out [128 m, 512 o] bf16 per m-block
                sq_sb = []
                for l in range(8):
                    c_ps = ps512.tile([128, 512], F32, name="c_ps", tag="mm512")
                    first = True
                    for kt in range(4):
                        for tap in range(4):
                            nc.tensor.matmul(
                                c_ps[:],
                                xp4[kt][:, 128 * l : 128 * (l + 1), tap],
                                wconv_sb[kt][:, 512 * tap : 512 * (tap + 1)],
                                start=first,
                                stop=False,
                            )
                            first = False
                    # += ones (x) bconv  (rank-1 bias over m partitions)
                    nc.tensor.matmul(
                        c_ps[:], crow(ROW_ONES, 128), crow(ROW_BCONV),
                        start=False, stop=True,
                    )
                    xcl = persist.tile([128, 512], BF16, name=f"xcl{l}", tag=f"xcl{l}")
                    nc.scalar.activation(xcl[:], c_ps[:], AF.Copy)
                    sq = persist.tile([128, 512], BF16, name=f"sq{l}", tag=f"sql{l}")
                    nc.vector.tensor_mul(sq[:], xcl[:], xcl[:])
                    xcl_sb.append(xcl)
                    sq_sb.append(sq)
                sx_ps = ps512.tile([1, 512], F32, name="sx_ps", tag="mm512")
                sq_ps = ps512.tile([1, 512], F32, name="sq_ps", tag="mm512")
                for l in range(8):
                    nc.tensor.matmul(
                        sx_ps[:], onec_sb[:], xcl_sb[l][:],
                        start=(l == 0), stop=(l == 7),
                    )
                    nc.tensor.matmul(
                        sq_ps[:], onec_sb[:], sq_sb[l][:],
                        start=(l == 0), stop=(l == 7),
                    )
                srow_sb = persist.tile([1, 1024], F32, name="srow_sb", tag="srow")
                nc.vector.tensor_copy(srow_sb[0:1, 0:512], sx_ps[:])
                nc.vector.tensor_copy(srow_sb[0:1, 512:1024], sq_ps[:])
                sx_row = srow_sb[0:1, 0:512]
                sq_row = srow_sb[0:1, 512:1024]

                # ---------------- Q projection ----------------
                qt_sb = persist.tile([128, N], BF16, name="qt_sb", tag="qt")
                for ch in range(8):
                    q_ps = ps512.tile([128, 512], F32, name="q_ps", tag="mm512")
                    for k in range(4):
                        nc.tensor.matmul(
                            q_ps[:],
                            wq_sb[k][:],
                            xp4[k][:, 128 * ch : 128 * (ch + 1), :],
                            start=(k == 0),
                            stop=(k == 3),
                        )
                    nc.vector.tensor_scalar_add(
                        qt_sb[:, 512 * ch : 512 * (ch + 1)], q_ps[:], bq_col
                    )

                # ---------------- stats math ----------------
                murow = stage.tile([1, 512], F32, name="murow", tag="murow", bufs=1)
                nc.vector.tensor_scalar_mul(murow[:], sx_row, 1.0 / M)
                negmu16 = persist.tile([1, 512], BF16, name="negmu16", tag="negmu")
                nc.vector.tensor_scalar_mul(negmu16[:], sx_row, -1.0 / M)
                verow = persist.tile([1, 512], F32, name="verow", tag="verow")
                nc.vector.tensor_scalar(
                    out=verow[:], in0=sq_row,
                    scalar1=1.0 / M, scalar2=EPS, op0=OP.mult, op1=OP.add,
                )
                mu2 = stage.tile([1, 512], F32, name="mu2", tag="mu2", bufs=1)
                nc.vector.tensor_mul(mu2[:], murow[:], murow[:])
                nc.vector.tensor_sub(verow[:], verow[:], mu2[:])
                # sqrtve row (bf16) = exp(0.5 ln ve)
                lrow = stage.tile([1, 512], F32, name="lrow", tag="lrow", bufs=1)
                nc.scalar.activation(lrow[:], verow[:], AF.Ln)
                sqve16 = persist.tile([1, 512], BF16, name="sqve16", tag="sqve")
                nc.scalar.activation(sqve16[:], lrow[:], AF.Exp, scale=0.5)
                # columns: ve -> [128, 4] via PE transpose, then exp/ln scales
                vecol_ps = ps512.tile([128, 4], F32, name="vecol_ps", tag="mm512")
                for j in range(4):
                    nc.tensor.transpose(
                        vecol_ps[:, j : j + 1],
                        verow[:, 128 * j : 128 * (j + 1)],
                        eyef_sb[0:1, 0:1],
                    )
                lcol = stage.tile([128, 4], F32, name="lcol", tag="lcol", bufs=1)
                nc.scalar.activation(lcol[:], vecol_ps[:], AF.Ln)
                esc_col = persist.tile([128, 4], F32, name="esc_col", tag="esc")
                nc.scalar.activation(esc_col[:], lcol[:], AF.Exp, scale=-0.5, bias=ln8_sb[:])
                vsc_col = persist.tile([128, 4], F32, name="vsc_col", tag="vsc")
                nc.scalar.activation(vsc_col[:], lcol[:], AF.Exp, scale=-0.5)

                # ---------------- KV ----------------
                kT_sb = persist.tile([128, 512], BF16, name="kT_sb", tag="kT")
                vT_sb = persist.tile([128, 512], BF16, name="vT_sb", tag="vT")
                for which, lo, t_row, s_row, dst in (
                    ("k", 0, ROW_TK, ROW_SK, kT_sb),
                    ("v", 128, ROW_TV, ROW_SV, vT_sb),
                ):
                    kv_ps = ps512.tile([128, 512], F32, name="kv_ps", tag="mm512")
                    for k in range(8):
                        nc.tensor.matmul(
                            kv_ps[:], wkv_sb[k][:, lo : lo + 128], xcl_sb[k][:],
                            start=(k == 0), stop=False,
                        )
                    nc.tensor.matmul(
                        kv_ps[:], crow(t_row, 128), negmu16[:],
                        start=False, stop=False,
                    )
                    nc.tensor.matmul(
                        kv_ps[:], crow(s_row, 128), sqve16[:],
                        start=False, stop=True,
                    )
                    nc.scalar.activation(dst[:], kv_ps[:], AF.Copy)

                # vaug[p][mt]: [128 c, 64 d] bf16, v^T with rs folded
                vaug_sb = []
                for p in range(2):
                    row = []
                    for mt in range(4):
                        t = persist.tile(
                            [128, 65], BF16, name=f"vaug{p}_{mt}", tag=f"va{p}{mt}"
                        )
                        tp = pstp.tile([128, 64], BF16, name="tpv", tag="ptp")
                        nc.tensor.transpose(
                            tp[:],
                            vT_sb[64 * p : 64 * (p + 1), 128 * mt : 128 * (mt + 1)],
                            eye_sb[64 * p : 64 * (p + 1), 64 * p : 64 * (p + 1)],
                        )
                        nc.vector.tensor_scalar_mul(
                            t[:, 0:64], tp[:], vsc_col[:, mt : mt + 1]
                        )
                        nc.vector.memset(t[:, 64:65], 1.0)
                        row.append(t)
                    vaug_sb.append(row)

                # ---------------- attention ----------------
                out2dT = []
                for p in range(2):
                    o_row = []
                    for ct in range(4):
                        t = persist.tile(
                            [128, 512], BF16, name=f"o2dT{p}_{ct}", tag=f"o2{p}{ct}"
                        )
                        o_row.append(t)
                    out2dT.append(o_row)

                def emit_s(ch, phat_all):
                    phat = {0: [], 1: []}
                    for mt in range(4):
                        for p in range(2):
                            s_ps = ps512.tile([128, 512], F32, name="s_ps", tag="mm512")
                            nc.tensor.matmul(
                                s_ps[:],
                                kT_sb[64 * p : 64 * (p + 1), 128 * mt : 128 * (mt + 1)],
                                qt_sb[64 * p : 64 * (p + 1), 512 * ch : 512 * (ch + 1)],
                                start=True,
                                stop=True,
                            )
                            ph = stage.tile(
                                [128, 512], BF16, name="phat", tag="phat", bufs=16
                            )
                            nc.scalar.activation(
                                ph[:], s_ps[:], AF.Exp, scale=esc_col[:, mt : mt + 1]
                            )
                            phat[p].append(ph)
                    phat_all[ch] = phat

                def emit_av(ch, phat_all):
                    phat = phat_all.pop(ch)
                    for p in range(2):
                        av2 = psa2.tile([128, 260], F32, name="av2", tag="psa2")
                        av3 = av2.rearrange("p (ct u) -> p ct u", ct=4, u=65)
                        for ct in range(4):
                            for mt in range(4):
                                nc.tensor.matmul(
                                    av3[:, ct, :],
                                    phat[p][mt][:, 128 * ct : 128 * (ct + 1)],
                                    vaug_sb[p][mt][:],
                                    start=(mt == 0),
                                    stop=(mt == 3),
                                )
                        rc = stage.tile([128, 4], F32, name="rc", tag="rc", bufs=4)
                        nc.vector.reciprocal(rc[:], av3[:, :, 64])
                        for ct in range(4):
                            o3 = out2dT[p][ct].rearrange(
                                "p (d s) -> p d s", d=64, s=8
                            )
                            nc.vector.tensor_scalar_mul(
                                o3[:, :, ch], av3[:, ct, 0:64], rc[:, ct : ct + 1]
                            )

                # software pipeline: S/exp of chunk ch+1 is emitted before
                # AV of chunk ch so the in-order PE never sits on an AV
                # matmul waiting for exp to drain.
                phat_all = {}
                emit_s(0, phat_all)
                for ch in range(8):
                    if ch + 1 < 8:
                        emit_s(ch + 1, phat_all)
                    emit_av(ch, phat_all)

                # ---------------- projection + output ----------------
                for p in range(2):
                    for rt in range(4):
                        pr_ps = ps512.tile([128, 512], F32, name="pr_ps", tag="mm512")
                        for ct in range(4):
                            nc.tensor.matmul(
                                pr_ps[:],
                                out2dT[p][ct][:, 128 * rt : 128 * (rt + 1)],
                                wproj_sb[ct][:],
                                start=(ct == 0),
                                stop=False,
                            )
                        nc.tensor.matmul(
                            pr_ps[:], crow(ROW_ONES, 128), crow(ROW_BPROJ),
                            start=False, stop=True,
                        )
                        of = stage.tile([128, 512], F32, name="of", tag="of", bufs=3)
                        nc.scalar.activation(of[:], pr_ps[:], AF.Copy)
                        r0 = 512 * p + 128 * rt
                        nc.sync.dma_start(out[r0 : r0 + 128, :], of[:])

    nc.compile()
    return nc


_NC_CACHE = None


def _get_module():
    global _NC_CACHE
    if _NC_CACHE is None:
        _NC_CACHE = _build_module()
    return _NC_CACHE


def _prep_core_inputs(inputs):
    """Host-side sharding: layout/permute/cast weights, build 8 in_maps."""
    x = np.asarray(inputs["x"], np.float32)
    Wq = np.asarray(inputs["Wq"], np.float32)
    bq = np.asarray(inputs["bq"], np.float32)
    Wconv = np.asarray(inputs["Wconv"], np.float32)
    bconv = np.asarray(inputs["bconv"], np.float32)
    gamma = np.asarray(inputs["gamma"], np.float32)
    beta = np.asarray(inputs["beta"], np.float32)
    Wkv = np.asarray(inputs["Wkv"], np.float32)
    bkv = np.asarray(inputs["bkv"], np.float32)
    Wproj = np.asarray(inputs["Wproj"], np.float32)
    bproj = np.asarray(inputs["bproj"], np.float32)

    # Xp: [ic, (m, tap)]; n = 128i + 64di + 2j + dj, m = 32i+j, tap = 2di+dj
    xp_g = []
    for b in range(B):
        xt = x[b].T.reshape(C, 32, 2, 32, 2)  # [ic, i, di, j, dj]
        xt = np.ascontiguousarray(
            xt.transpose(0, 1, 3, 2, 4).reshape(C, 8, 512)  # [ic, blk, rest]
        )
        xp_g.append(xt)

    wconvt = np.ascontiguousarray(
        Wconv.transpose(1, 2, 3, 0).reshape(C, 4, C)
    ).astype(NP_BF16)

    wkvp = gamma[:, None] * Wkv
    s_full = beta @ Wkv + bkv

    # Wproj row permutation: u' = 128 i' + 4 j + 2 di + dj -> n' = 128 i' + 64 di + 2 j + dj
    up = np.arange(C)
    i_, j_ = up // 128, (up % 128) // 4
    di, dj = (up % 4) // 2, up % 2
    nprime = 128 * i_ + 64 * di + 2 * j_ + dj
    wproj_perm = np.ascontiguousarray(Wproj[nprime, :]).astype(NP_BF16)

    eye128 = np.eye(128, dtype=np.float32).astype(NP_BF16)
    eyef = np.eye(8, dtype=np.float32)

    in_maps = []
    for core in range(N_CORES):
        b, g = divmod(core, 4)
        kcols = slice(128 * g, 128 * (g + 1))
        vcols = slice(512 + 128 * g, 512 + 128 * (g + 1))
        xp_loc = np.ascontiguousarray(xp_g[b].reshape(C, N)).astype(NP_BF16)
        small = np.zeros((128, 2), np.float32)
        small[:, 0] = bq[kcols]
        rows16 = np.zeros((8, C), np.float32)
        rows16[ROW_BCONV] = bconv
        rows16[ROW_BPROJ] = bproj
        rows16[ROW_TK, 0:128] = wkvp[:, kcols].sum(0)
        rows16[ROW_TV, 0:128] = wkvp[:, vcols].sum(0)
        rows16[ROW_SK, 0:128] = s_full[kcols]
        rows16[ROW_SV, 0:128] = s_full[vcols]
        rows16[ROW_ONES] = 1.0
        wkv2 = np.concatenate([wkvp[:, kcols], wkvp[:, vcols]], axis=1)
        in_maps.append(
            {
                "xp": xp_loc,
                "wq": np.ascontiguousarray(Wq[:, kcols]).astype(NP_BF16),
                "wconvt": wconvt,
                "wkv2": np.ascontiguousarray(wkv2).astype(NP_BF16),
                "wproj": wproj_perm,
                "small": small,
                "rows16": rows16.reshape(1, 8 * C).astype(NP_BF16),
                "eye128": eye128,
                "eyef": eyef,
            }
        )
    return in_maps


def run_spmd(inputs, **kwargs):
    """Run the SPMD kernel; returns (full_output, BassKernelResults)."""
    nc = _get_module()
    in_maps = _prep_core_inputs(inputs)
    res = run_bass_kernel_spmd(nc, in_maps, core_ids=list(range(N_CORES)), **kwargs)
    full = np.empty((B, N, C), np.float32)
    for core in range(N_CORES):
        b, g = divmod(core, 4)
        full[b, 1024 * g : 1024 * (g + 1), :] = res.results[core]["out"]
    return full, res


def kernel(**inputs) -> np.ndarray:
    full, _ = run_spmd(inputs)
    return full
